# revision 14
# baseline (speedup 1.0000x reference)
"""ConstrainedMLP Trainium2 kernel.

Strategy
--------
Rows are sharded BY GROUP (the quantile groups of `group_mask`): there are
exactly 8 groups and 8 cores, so each core owns one group and the
constrained-projection step becomes fully core-local (no collectives).

Per core:
  MLP   : h1 = relu(x@W1+b1), h2 = relu(h1@W2+b2), y_u = h2@Wf+bf
          computed in "transposed activation" layout (h1T/h2T = [feat, rows])
          with fp32r matmuls so every matmul has a 512-wide moving operand.
  Proj  : the reference's 300-iteration Dykstra projection converges to the
          exact QP solution  z* = clip(z0 + lam/w, +-EPS)  with a single
          scalar lam per group chosen so the group mean of z* hits
          clip(mean(clip(z0)), +-DELTA).  lam is found with an 8-round
          8-ary search (7 candidates per round, 3 bits/round); all search
          state is replicated across the 128 partitions so each round is a
          short DVE chain plus one all-ones matmul for the cross-partition
          reduction.  (Validated vs the 300-iter reference: ~2e-6.)

Self-contained: only numpy + the concourse/bass runtime installed in the
environment. No files are read.
"""

import os
import numpy as np

EPS = 0.15
DELTA = 0.05
W0 = 64.0        # initial lambda interval: [-32, 32]
N_ROUNDS = 7     # 8-ary search rounds -> final width 64/8^7 ~ 3.1e-5
NCAND = 7        # candidates per round
P = 128          # SBUF partitions
BLK = 512        # row block (moving-operand width)

_PROGRAM_CACHE = {}
LAST_RESULT = None  # test harness introspection (exec_time etc.)


def _build_program(D, H1, H2, R, nrows, bf_val):
    import concourse.bass as bass
    import concourse.tile as tile
    from concourse import bacc, mybir
    from contextlib import ExitStack

    f32 = mybir.dt.float32
    f32r = mybir.dt.float32r
    Alu = mybir.AluOpType
    Act = mybir.ActivationFunctionType

    KD = D // P       # contraction chunks for layer 1
    K1 = H1 // P      # h1 feature chunks
    K2 = H2 // P      # h2 feature chunks
    NB = R // BLK     # row blocks
    C = R // P        # columns of the [128, C] projection layout
    fast_scatter = (BLK % C == 0)
    PPB = BLK // C if fast_scatter else 0  # partitions covered per row block
    no_pad = (nrows == R)

    nc = bacc.Bacc("TRN2", target_bir_lowering=False, debug=False, num_devices=8)

    xt = nc.dram_tensor("xt", [D, R], f32, kind="ExternalInput").ap()
    w1 = nc.dram_tensor("w1", [D, H1], f32, kind="ExternalInput").ap()
    w2 = nc.dram_tensor("w2", [H1, H2], f32, kind="ExternalInput").ap()
    wf = nc.dram_tensor("wf", [H2], f32, kind="ExternalInput").ap()
    b1 = nc.dram_tensor("b1", [H1], f32, kind="ExternalInput").ap()
    b2 = nc.dram_tensor("b2", [H2], f32, kind="ExternalInput").ap()
    c2d = nc.dram_tensor("c2d", [P, C], f32, kind="ExternalInput").ap()
    ci2d = nc.dram_tensor("ci2d", [P, C], f32, kind="ExternalInput").ap()
    d2d = nc.dram_tensor("d2d", [P, C], f32, kind="ExternalInput").ap()
    a2d = nc.dram_tensor("a2d", [P, C], f32, kind="ExternalInput").ap()
    out2d = nc.dram_tensor("out2d", [P, C], f32, kind="ExternalOutput").ap()

    with tile.TileContext(nc) as tc, ExitStack() as ctx:
        consts = ctx.enter_context(tc.tile_pool(name="consts", bufs=1))
        xpool = ctx.enter_context(tc.tile_pool(name="xp", bufs=6))
        h1pool = ctx.enter_context(tc.tile_pool(name="h1p", bufs=3))
        h2pool = ctx.enter_context(tc.tile_pool(name="h2p", bufs=3))
        ytpool = ctx.enter_context(tc.tile_pool(name="ytp", bufs=2))
        ps1 = ctx.enter_context(tc.tile_pool(name="ps1", bufs=3, space="PSUM"))
        ps2 = ctx.enter_context(tc.tile_pool(name="ps2", bufs=3, space="PSUM"))
        psy = ctx.enter_context(tc.tile_pool(name="psy", bufs=1, space="PSUM"))
        pst = ctx.enter_context(tc.tile_pool(name="pst", bufs=1, space="PSUM"))
        proj = ctx.enter_context(tc.tile_pool(name="proj", bufs=1))
        if not fast_scatter:
            dramp = ctx.enter_context(tc.tile_pool(name="dramp", bufs=1,
                                                   space="DRAM"))
            yt_dram = dramp.tile([R], f32, name="yt_dram")

        # ---- resident weights / constants ----
        # Single ordered HWDGE stream arranged so the first L1 matmuls can
        # start as early as possible: w1[0], block-0 x tiles, w1[1:], b1,
        # then layer-2 / projection constants, then the b>=1 x stream.
        w1sb = [consts.tile([P, H1], f32r, tag=f"w1_{k}", name=f"w1sb{k}")
                for k in range(KD)]
        xts0 = [xpool.tile([P, BLK], f32r, tag=f"x{k}", name=f"x0_{k}")
                for k in range(KD)]
        for k in range(KD):
            nc.sync.dma_start(out=w1sb[k], in_=w1[k * P:(k + 1) * P, :].bitcast(f32r))
            nc.sync.dma_start(out=xts0[k], in_=xt[k * P:(k + 1) * P, 0:BLK].bitcast(f32r))
        b1sb = consts.tile([P, K1], f32, tag="b1")
        nc.sync.dma_start(out=b1sb, in_=b1.rearrange("(m p) -> p m", p=P))

        w2sb = []
        for k in range(K1):
            t = consts.tile([P, H2], f32r, tag=f"w2_{k}")
            nc.sync.dma_start(out=t, in_=w2[k * P:(k + 1) * P, :].bitcast(f32r))
            w2sb.append(t)
        wfsb = consts.tile([P, K2], f32r, tag="wf")
        nc.sync.dma_start(out=wfsb, in_=wf.rearrange("(k p) -> p k", p=P).bitcast(f32r))
        b2sb = consts.tile([P, K2], f32, tag="b2")
        nc.sync.dma_start(out=b2sb, in_=b2.rearrange("(m p) -> p m", p=P))

        ci_sb = consts.tile([P, C], f32, tag="ci_sb")
        nc.gpsimd.dma_start(out=ci_sb, in_=ci2d)
        c_sb = consts.tile([P, C], f32, tag="c_sb")
        nc.gpsimd.dma_start(out=c_sb, in_=c2d)
        d_sb = consts.tile([P, C], f32, tag="d_sb")
        nc.gpsimd.dma_start(out=d_sb, in_=d2d)
        a_sb = consts.tile([P, C], f32, tag="a_sb")
        nc.gpsimd.dma_start(out=a_sb, in_=a2d)

        onesmat = consts.tile([P, P], f32, tag="onesmat")
        nc.vector.memset(onesmat, 1.0)
        iota = consts.tile([P, NCAND], f32, tag="iota")
        for j in range(NCAND):
            nc.vector.memset(iota[:, j:j + 1], float(j + 1))

        y2 = consts.tile([P, C], f32, tag="y2")
        z0 = proj.tile([P, C], f32, tag="z0")
        tmp = proj.tile([P, C], f32, tag="tmp")
        red = proj.tile([P, 1], f32, tag="red")

        # ---- MLP over row blocks ----
        for b in range(NB):
            cols = slice(b * BLK, (b + 1) * BLK)
            if b == 0:
                xts = xts0
            else:
                xts = []
                for k in range(KD):
                    t = xpool.tile([P, BLK], f32r, tag=f"x{k}", name=f"x{b}_{k}")
                    nc.sync.dma_start(out=t, in_=xt[k * P:(k + 1) * P, cols].bitcast(f32r))
                    xts.append(t)

            h1t = h1pool.tile([P, K1, BLK], f32r, tag="h1t")
            for m in range(K1):
                pt = ps1.tile([P, BLK], f32, tag="ps1")
                for k in range(KD):
                    nc.tensor.matmul(
                        pt,
                        lhsT=w1sb[k][:, m * P:(m + 1) * P],
                        rhs=xts[k][:, :],
                        start=(k == 0),
                        stop=(k == KD - 1),
                    )
                nc.scalar.activation(
                    out=h1t[:, m, :], in_=pt, func=Act.Relu,
                    bias=b1sb[:, m:m + 1], scale=1.0,
                )

            h2t = h2pool.tile([P, K2, BLK], f32r, tag="h2t")
            for m in range(K2):
                pt = ps2.tile([P, BLK], f32, tag="ps2")
                for k in range(K1):
                    nc.tensor.matmul(
                        pt,
                        lhsT=w2sb[k][:, m * P:(m + 1) * P],
                        rhs=h1t[:, k, :],
                        start=(k == 0),
                        stop=(k == K1 - 1),
                    )
                # relu+bias on the vector engine to keep ScalarE headroom
                nc.vector.tensor_scalar(
                    out=h2t[:, m, :], in0=pt, scalar1=b2sb[:, m:m + 1],
                    scalar2=0.0, op0=Alu.add, op1=Alu.max,
                )

            # final layer: yT[1, BLK] = sum_k wf_k.T @ h2t_k  (fp32r, N=512)
            pty = psy.tile([1, BLK], f32, tag="psy")
            for k in range(K2):
                nc.tensor.matmul(
                    pty,
                    lhsT=wfsb[:, k:k + 1],
                    rhs=h2t[:, k, :],
                    start=(k == 0),
                    stop=(k == K2 - 1),
                )
            ytb = ytpool.tile([1, BLK], f32, tag="ytb")
            nc.scalar.activation(out=ytb, in_=pty, func=Act.Copy,
                                 bias=float(bf_val), scale=1.0)
            # scatter rows b*BLK..(b+1)*BLK into y2[p, c] with r = p*C + c
            if fast_scatter:
                nc.sync.dma_start(out=y2[b * PPB:(b + 1) * PPB, :],
                                  in_=ytb[0:1, :])
            else:
                nc.sync.dma_start(out=yt_dram[b * BLK:(b + 1) * BLK],
                                  in_=ytb[0:1, :])
            # every 4 blocks (engine partition offsets must be 32-aligned):
            # z0 = y*cinv - 1 and tau partial sums on the vector engine,
            # which is idle during the MLP — hides the projection preamble.
            if fast_scatter and (b + 1) * PPB % 32 == 0:
                pp = slice((b + 1) * PPB - 32, (b + 1) * PPB)
                nc.vector.tensor_tensor(out=z0[pp, :], in0=y2[pp, :],
                                        in1=ci_sb[pp, :], op=Alu.mult)
                nc.vector.tensor_scalar(out=z0[pp, :], in0=z0[pp, :],
                                        scalar1=-1.0, scalar2=None, op0=Alu.add)
                nc.vector.tensor_scalar(out=tmp[pp, :], in0=z0[pp, :],
                                        scalar1=EPS, scalar2=-EPS,
                                        op0=Alu.min, op1=Alu.max)
                if not no_pad:
                    nc.vector.tensor_tensor(out=tmp[pp, :], in0=tmp[pp, :],
                                            in1=a_sb[pp, :], op=Alu.mult)
                nc.vector.tensor_reduce(out=red[pp, :], in_=tmp[pp, :],
                                        axis=mybir.AxisListType.X, op=Alu.add)

        # ---- projection ----
        # z* = clip(z0 + lam*d, +-EPS) with scalar lam s.t.
        #   S(lam) := sum_r m_r clip(z0_r + lam*d_r)  ==  taun,
        #   taun = clip(S(0), +-DELTA*n)   (m absorbed in a2d when padded;
        #                                   m == 1 and plain sums when not)
        # 8-ary search over lam; state replicated across partitions so each
        # round is one DVE chain + one all-ones matmul (partition reduce).
        tmp3 = proj.tile([P, NCAND, C], f32, tag="tmp3")
        red7 = proj.tile([P, NCAND], f32, tag="red7")
        lamc = proj.tile([P, NCAND], f32, tag="lamc")
        predl = proj.tile([P, NCAND], f32, tag="predl")
        cnt = proj.tile([P, 1], f32, tag="cnt")
        taun = proj.tile([P, 1], f32, tag="taun")
        los = [proj.tile([P, 1], f32, tag=f"lo{i}", name=f"lo{i}")
               for i in range(2)]

        z0b = z0.rearrange("p (o c) -> p o c", o=1).to_broadcast([P, NCAND, C])
        d_b = d_sb.rearrange("p (o c) -> p o c", o=1).to_broadcast([P, NCAND, C])
        a_b = a_sb.rearrange("p (o c) -> p o c", o=1).to_broadcast([P, NCAND, C])

        if not fast_scatter:
            # general path: gather y back, then z0 / tau sums in one shot
            nc.sync.dma_start(out=y2, in_=yt_dram.rearrange("(p c) -> p c", p=P))
            nc.vector.tensor_tensor(out=z0, in0=y2, in1=ci_sb, op=Alu.mult)
            nc.vector.tensor_scalar(out=z0, in0=z0, scalar1=-1.0, scalar2=None,
                                    op0=Alu.add)
            nc.vector.tensor_scalar(out=tmp, in0=z0, scalar1=EPS, scalar2=-EPS,
                                    op0=Alu.min, op1=Alu.max)
            if not no_pad:
                nc.vector.tensor_tensor(out=tmp, in0=tmp, in1=a_sb, op=Alu.mult)
            nc.vector.tensor_reduce(out=red, in_=tmp,
                                    axis=mybir.AxisListType.X, op=Alu.add)
        # taun = clip(S(0), +-DELTA*n): partials in red were accumulated
        # per block during the MLP; finish with the partition reduce.
        t0ps = pst.tile([P, NCAND], f32, tag="hps", name="t0ps")
        nc.tensor.matmul(t0ps[:, 0:1], lhsT=onesmat, rhs=red, start=True,
                         stop=True)
        dn = DELTA * (float(nrows) if no_pad else 1.0)
        nc.vector.tensor_scalar(out=taun, in0=t0ps[:, 0:1], scalar1=dn,
                                scalar2=-dn, op0=Alu.min, op1=Alu.max)

        nc.vector.memset(los[0], -W0 / 2.0)
        lo = los[0]
        width = W0
        for r_i in range(N_ROUNDS):
            step = width / (NCAND + 1)
            # candidate lambdas: lo + j*step, j = 1..NCAND
            nc.vector.tensor_scalar(out=lamc, in0=iota, scalar1=step,
                                    scalar2=lo[:, 0:1], op0=Alu.mult,
                                    op1=Alu.add)
            lam_b = lamc.rearrange("p (o j) -> p j o", o=1).to_broadcast(
                [P, NCAND, C])
            nc.vector.tensor_tensor(out=tmp3, in0=d_b, in1=lam_b, op=Alu.mult)
            nc.vector.tensor_tensor(out=tmp3, in0=tmp3, in1=z0b, op=Alu.add)
            nc.vector.tensor_scalar(out=tmp3, in0=tmp3, scalar1=EPS,
                                    scalar2=-EPS, op0=Alu.min, op1=Alu.max)
            if not no_pad:
                nc.vector.tensor_tensor(out=tmp3, in0=tmp3, in1=a_b,
                                        op=Alu.mult)
            nc.vector.tensor_reduce(out=red7, in_=tmp3,
                                    axis=mybir.AxisListType.X, op=Alu.add)
            hps = pst.tile([P, NCAND], f32, tag="hps", name=f"hps{r_i}")
            nc.tensor.matmul(hps, lhsT=onesmat, rhs=red7, start=True,
                             stop=True)
            nc.vector.tensor_scalar(out=predl, in0=hps, scalar1=taun[:, 0:1],
                                    scalar2=None, op0=Alu.is_le)
            nc.vector.tensor_reduce(out=cnt, in_=predl,
                                    axis=mybir.AxisListType.X, op=Alu.add)
            lo_next = los[(r_i + 1) % 2]
            nc.vector.tensor_scalar(out=lo_next, in0=cnt, scalar1=step,
                                    scalar2=lo[:, 0:1], op0=Alu.mult,
                                    op1=Alu.add)
            lo = lo_next
            width = step  # interval shrinks to one candidate slot

        # final: lam = lo + width/2 ; out = (clip(z0 + lam*d) + 1) * c
        lamf = proj.tile([P, 1], f32, tag="lamf")
        nc.vector.tensor_scalar(out=lamf, in0=lo, scalar1=width / 2.0,
                                scalar2=None, op0=Alu.add)
        nc.vector.tensor_scalar(out=tmp, in0=d_sb, scalar1=lamf[:, 0:1],
                                scalar2=None, op0=Alu.mult)
        nc.vector.tensor_tensor(out=tmp, in0=tmp, in1=z0, op=Alu.add)
        nc.vector.tensor_scalar(out=tmp, in0=tmp, scalar1=EPS, scalar2=-EPS,
                                op0=Alu.min, op1=Alu.max)
        nc.vector.tensor_scalar(out=tmp, in0=tmp, scalar1=1.0, scalar2=None,
                                op0=Alu.add)
        nc.vector.tensor_tensor(out=tmp, in0=tmp, in1=c_sb, op=Alu.mult)
        nc.sync.dma_start(out=out2d, in_=tmp)

    nc.compile()
    return nc


def kernel(**inputs):
    global LAST_RESULT
    x = np.ascontiguousarray(np.asarray(inputs["x"], dtype=np.float32))
    W1 = np.ascontiguousarray(np.asarray(inputs["W1"], dtype=np.float32))
    b1 = np.ascontiguousarray(np.asarray(inputs["b1"], dtype=np.float32))
    W2 = np.ascontiguousarray(np.asarray(inputs["W2"], dtype=np.float32))
    b2 = np.ascontiguousarray(np.asarray(inputs["b2"], dtype=np.float32))
    Wf = np.ascontiguousarray(np.asarray(inputs["Wf"], dtype=np.float32))
    bf = float(np.asarray(inputs["bf"], dtype=np.float32).reshape(-1)[0])
    c = np.ascontiguousarray(np.asarray(inputs["constraint_constant"], dtype=np.float32))
    gm = np.asarray(inputs["group_mask"], dtype=np.float32)

    N, D = x.shape
    H1 = W1.shape[1]
    H2 = W2.shape[1]
    G = gm.shape[0]
    assert G == 8, "this kernel shards one quantile group per core"
    assert D % P == 0 and H1 % P == 0 and H2 % P == 0 and Wf.shape[1] == 1

    g = np.argmax(gm, axis=0)
    sizes = np.bincount(g, minlength=G)
    R = int(-(-sizes.max() // BLK) * BLK)   # per-core padded rows
    C = R // P
    uniform = bool((sizes == sizes[0]).all() and sizes[0] == R)

    order = np.argsort(g, kind="stable")
    bounds = np.zeros(G + 1, np.int64)
    np.cumsum(sizes, out=bounds[1:])

    def to2d(vec):
        # local row r -> (p = r // C, col = r % C)
        return np.ascontiguousarray(vec.reshape(P, C))

    xT = x.T  # [D, N] view
    in_maps = []
    rows_per_core = []
    for j in range(G):
        rows = order[bounds[j]:bounds[j + 1]]
        nrows = rows.shape[0]
        rows_per_core.append(rows)

        xtj = np.zeros((D, R), np.float32)
        np.take(xT, rows, axis=1, out=xtj[:, :nrows])

        cj = np.ones(R, np.float32)
        cj[:nrows] = c[rows]
        cij = 1.0 / cj
        dj = np.zeros(R, np.float32)
        dj[:nrows] = cij[:nrows] * cij[:nrows]
        aj = np.zeros(R, np.float32)
        aj[:nrows] = 1.0 / nrows

        in_maps.append(dict(
            xt=xtj, w1=W1, w2=W2, wf=Wf.reshape(-1), b1=b1, b2=b2,
            c2d=to2d(cj), ci2d=to2d(cij), d2d=to2d(dj), a2d=to2d(aj),
        ))

    nrows_build = R if uniform else -1   # -1 -> general padded path
    key = (D, H1, H2, R, nrows_build, float(bf))
    nc = _PROGRAM_CACHE.get(key)
    if nc is None:
        nc = _build_program(D, H1, H2, R, nrows_build, float(bf))
        _PROGRAM_CACHE[key] = nc

    from concourse.bass_utils import run_bass_kernel_spmd
    trace = bool(int(os.environ.get("KERNEL_PROFILE", "0")))
    res = run_bass_kernel_spmd(nc, in_maps, list(range(G)), trace=trace)
    LAST_RESULT = res

    out = np.empty((N, 1), np.float32)
    for j in range(G):
        y2d = res.results[j]["out2d"]          # [128, C], row r = p*C + col
        yvec = y2d.reshape(-1)
        out[rows_per_core[j], 0] = yvec[:rows_per_core[j].shape[0]]
    return out


# revision 16
# speedup vs baseline: 1.0019x; 1.0019x over previous
"""ConstrainedMLP Trainium2 kernel.

Strategy
--------
Rows are sharded BY GROUP (the quantile groups of `group_mask`): there are
exactly 8 groups and 8 cores, so each core owns one group and the
constrained-projection step becomes fully core-local (no collectives).

Per core:
  MLP   : h1 = relu(x@W1+b1), h2 = relu(h1@W2+b2), y_u = h2@Wf+bf
          computed in "transposed activation" layout (h1T/h2T = [feat, rows])
          with fp32r matmuls so every matmul has a 512-wide moving operand.
  Proj  : the reference's 300-iteration Dykstra projection converges to the
          exact QP solution  z* = clip(z0 + lam/w, +-EPS)  with a single
          scalar lam per group chosen so the group mean of z* hits
          clip(mean(clip(z0)), +-DELTA).  lam is found with a 7-round
          8-ary search (7 candidates per round, 3 bits/round); all search
          state is replicated across the 128 partitions so each round is a
          short DVE chain plus one all-ones matmul for the cross-partition
          reduction.  (Validated vs the 300-iter reference: ~2e-6.)

Self-contained: only numpy + the concourse/bass runtime installed in the
environment. No files are read.
"""

import os
import numpy as np

EPS = 0.15
DELTA = 0.05
W0 = 64.0        # initial lambda interval: [-32, 32]
N_ROUNDS = 7     # 8-ary search rounds -> final width 64/8^7 ~ 3.1e-5
NCAND = 7        # candidates per round
P = 128          # SBUF partitions
BLK = 512        # row block (moving-operand width)

_PROGRAM_CACHE = {}
LAST_RESULT = None  # test harness introspection (exec_time etc.)


def _build_program(D, H1, H2, R, nrows, bf_val):
    import concourse.bass as bass
    import concourse.tile as tile
    from concourse import bacc, mybir
    from contextlib import ExitStack

    f32 = mybir.dt.float32
    f32r = mybir.dt.float32r
    Alu = mybir.AluOpType
    Act = mybir.ActivationFunctionType

    KD = D // P       # contraction chunks for layer 1
    K1 = H1 // P      # h1 feature chunks
    K2 = H2 // P      # h2 feature chunks
    NB = R // BLK     # row blocks
    C = R // P        # columns of the [128, C] projection layout
    fast_scatter = (BLK % C == 0)
    PPB = BLK // C if fast_scatter else 0  # partitions covered per row block
    no_pad = (nrows == R)

    nc = bacc.Bacc("TRN2", target_bir_lowering=False, debug=False, num_devices=8)

    xt = nc.dram_tensor("xt", [D, R], f32, kind="ExternalInput").ap()
    w1 = nc.dram_tensor("w1", [D, H1], f32, kind="ExternalInput").ap()
    w2 = nc.dram_tensor("w2", [H1, H2], f32, kind="ExternalInput").ap()
    wf = nc.dram_tensor("wf", [H2], f32, kind="ExternalInput").ap()
    b1 = nc.dram_tensor("b1", [H1], f32, kind="ExternalInput").ap()
    b2 = nc.dram_tensor("b2", [H2], f32, kind="ExternalInput").ap()
    c2d = nc.dram_tensor("c2d", [P, C], f32, kind="ExternalInput").ap()
    ci2d = nc.dram_tensor("ci2d", [P, C], f32, kind="ExternalInput").ap()
    d2d = nc.dram_tensor("d2d", [P, C], f32, kind="ExternalInput").ap()
    a2d = nc.dram_tensor("a2d", [P, C], f32, kind="ExternalInput").ap()
    out2d = nc.dram_tensor("out2d", [P, C], f32, kind="ExternalOutput").ap()

    with tile.TileContext(nc) as tc, ExitStack() as ctx:
        consts = ctx.enter_context(tc.tile_pool(name="consts", bufs=1))
        xpool = ctx.enter_context(tc.tile_pool(name="xp", bufs=6))
        h1pool = ctx.enter_context(tc.tile_pool(name="h1p", bufs=3))
        h2pool = ctx.enter_context(tc.tile_pool(name="h2p", bufs=3))
        ytpool = ctx.enter_context(tc.tile_pool(name="ytp", bufs=2))
        ps1 = ctx.enter_context(tc.tile_pool(name="ps1", bufs=3, space="PSUM"))
        ps2 = ctx.enter_context(tc.tile_pool(name="ps2", bufs=3, space="PSUM"))
        psy = ctx.enter_context(tc.tile_pool(name="psy", bufs=1, space="PSUM"))
        pst = ctx.enter_context(tc.tile_pool(name="pst", bufs=1, space="PSUM"))
        proj = ctx.enter_context(tc.tile_pool(name="proj", bufs=1))
        if not fast_scatter:
            dramp = ctx.enter_context(tc.tile_pool(name="dramp", bufs=1,
                                                   space="DRAM"))
            yt_dram = dramp.tile([R], f32, name="yt_dram")

        # ---- resident weights / constants ----
        # Single ordered HWDGE stream arranged so the first L1 matmuls can
        # start as early as possible: w1[0], block-0 x tiles, w1[1:], b1,
        # then layer-2 / projection constants, then the b>=1 x stream.
        w1sb = [consts.tile([P, H1], f32r, tag=f"w1_{k}", name=f"w1sb{k}")
                for k in range(KD)]
        xts0 = [xpool.tile([P, BLK], f32r, tag=f"x{k}", name=f"x0_{k}")
                for k in range(KD)]
        for k in range(KD):
            nc.sync.dma_start(out=w1sb[k], in_=w1[k * P:(k + 1) * P, :].bitcast(f32r))
            nc.sync.dma_start(out=xts0[k], in_=xt[k * P:(k + 1) * P, 0:BLK].bitcast(f32r))
        b1sb = consts.tile([P, K1], f32, tag="b1")
        nc.sync.dma_start(out=b1sb, in_=b1.rearrange("(m p) -> p m", p=P))

        w2sb = []
        for k in range(K1):
            t = consts.tile([P, H2], f32r, tag=f"w2_{k}")
            nc.sync.dma_start(out=t, in_=w2[k * P:(k + 1) * P, :].bitcast(f32r))
            w2sb.append(t)
        wfsb = consts.tile([P, K2], f32r, tag="wf")
        nc.sync.dma_start(out=wfsb, in_=wf.rearrange("(k p) -> p k", p=P).bitcast(f32r))
        b2sb = consts.tile([P, K2], f32, tag="b2")
        nc.sync.dma_start(out=b2sb, in_=b2.rearrange("(m p) -> p m", p=P))

        ci_sb = consts.tile([P, C], f32, tag="ci_sb")
        nc.gpsimd.dma_start(out=ci_sb, in_=ci2d)
        c_sb = consts.tile([P, C], f32, tag="c_sb")
        nc.gpsimd.dma_start(out=c_sb, in_=c2d)
        d_sb = consts.tile([P, C], f32, tag="d_sb")
        nc.gpsimd.dma_start(out=d_sb, in_=d2d)
        a_sb = consts.tile([P, C], f32, tag="a_sb")
        nc.gpsimd.dma_start(out=a_sb, in_=a2d)

        onesmat = consts.tile([P, P], f32, tag="onesmat")
        nc.vector.memset(onesmat, 1.0)
        iota = consts.tile([P, NCAND], f32, tag="iota")
        for j in range(NCAND):
            nc.vector.memset(iota[:, j:j + 1], float(j + 1))

        y2 = consts.tile([P, C], f32, tag="y2")
        z0 = proj.tile([P, C], f32, tag="z0")
        tmp = proj.tile([P, C], f32, tag="tmp")
        red = proj.tile([P, 1], f32, tag="red")

        # ---- MLP over row blocks ----
        for b in range(NB):
            cols = slice(b * BLK, (b + 1) * BLK)
            if b == 0:
                xts = xts0
            else:
                xts = []
                for k in range(KD):
                    t = xpool.tile([P, BLK], f32r, tag=f"x{k}", name=f"x{b}_{k}")
                    nc.sync.dma_start(out=t, in_=xt[k * P:(k + 1) * P, cols].bitcast(f32r))
                    xts.append(t)

            h1t = h1pool.tile([P, K1, BLK], f32r, tag="h1t")
            for m in range(K1):
                pt = ps1.tile([P, BLK], f32, tag="ps1")
                for k in range(KD):
                    nc.tensor.matmul(
                        pt,
                        lhsT=w1sb[k][:, m * P:(m + 1) * P],
                        rhs=xts[k][:, :],
                        start=(k == 0),
                        stop=(k == KD - 1),
                    )
                nc.scalar.activation(
                    out=h1t[:, m, :], in_=pt, func=Act.Relu,
                    bias=b1sb[:, m:m + 1], scale=1.0,
                )

            h2t = h2pool.tile([P, K2, BLK], f32r, tag="h2t")
            for m in range(K2):
                pt = ps2.tile([P, BLK], f32, tag="ps2")
                for k in range(K1):
                    nc.tensor.matmul(
                        pt,
                        lhsT=w2sb[k][:, m * P:(m + 1) * P],
                        rhs=h1t[:, k, :],
                        start=(k == 0),
                        stop=(k == K1 - 1),
                    )
                # relu+bias on the vector engine to keep ScalarE headroom
                nc.vector.tensor_scalar(
                    out=h2t[:, m, :], in0=pt, scalar1=b2sb[:, m:m + 1],
                    scalar2=0.0, op0=Alu.add, op1=Alu.max,
                )

            # final layer: yT[1, BLK] = sum_k wf_k.T @ h2t_k  (fp32r, N=512)
            pty = psy.tile([1, BLK], f32, tag="psy")
            for k in range(K2):
                nc.tensor.matmul(
                    pty,
                    lhsT=wfsb[:, k:k + 1],
                    rhs=h2t[:, k, :],
                    start=(k == 0),
                    stop=(k == K2 - 1),
                )
            ytb = ytpool.tile([1, BLK], f32, tag="ytb")
            nc.scalar.activation(out=ytb, in_=pty, func=Act.Copy,
                                 bias=float(bf_val), scale=1.0)
            # scatter rows b*BLK..(b+1)*BLK into y2[p, c] with r = p*C + c
            if fast_scatter:
                nc.sync.dma_start(out=y2[b * PPB:(b + 1) * PPB, :],
                                  in_=ytb[0:1, :])
            else:
                nc.sync.dma_start(out=yt_dram[b * BLK:(b + 1) * BLK],
                                  in_=ytb[0:1, :])
            # every 4 blocks (engine partition offsets must be 32-aligned):
            # z0 = y*cinv - 1 and tau partial sums on the vector engine,
            # which is idle during the MLP — hides the projection preamble.
            if fast_scatter and (b + 1) * PPB % 32 == 0:
                pp = slice((b + 1) * PPB - 32, (b + 1) * PPB)
                nc.vector.tensor_tensor(out=z0[pp, :], in0=y2[pp, :],
                                        in1=ci_sb[pp, :], op=Alu.mult)
                nc.vector.tensor_scalar(out=z0[pp, :], in0=z0[pp, :],
                                        scalar1=-1.0, scalar2=None, op0=Alu.add)
                nc.vector.tensor_scalar(out=tmp[pp, :], in0=z0[pp, :],
                                        scalar1=EPS, scalar2=-EPS,
                                        op0=Alu.min, op1=Alu.max)
                if not no_pad:
                    nc.vector.tensor_tensor(out=tmp[pp, :], in0=tmp[pp, :],
                                            in1=a_sb[pp, :], op=Alu.mult)
                nc.vector.tensor_reduce(out=red[pp, :], in_=tmp[pp, :],
                                        axis=mybir.AxisListType.X, op=Alu.add)

        # ---- projection ----
        # z* = clip(z0 + lam*d, +-EPS) with scalar lam s.t.
        #   S(lam) := sum_r m_r clip(z0_r + lam*d_r)  ==  taun,
        #   taun = clip(S(0), +-DELTA*n)   (m absorbed in a2d when padded;
        #                                   m == 1 and plain sums when not)
        # 7-round 8-ary search over lam; state replicated across partitions,
        # each round = one DVE chain + one all-ones matmul (partition reduce).
        tmp3 = proj.tile([P, NCAND, C], f32, tag="tmp3")
        red7 = proj.tile([P, NCAND], f32, tag="red7")
        lamc = proj.tile([P, NCAND], f32, tag="lamc")
        predl = proj.tile([P, NCAND], f32, tag="predl")
        cnt = proj.tile([P, 1], f32, tag="cnt")
        taun = proj.tile([P, 1], f32, tag="taun")
        los = [proj.tile([P, 1], f32, tag=f"lo{i}", name=f"lo{i}")
               for i in range(2)]

        z0b = z0.rearrange("p (o c) -> p o c", o=1).to_broadcast([P, NCAND, C])
        d_b = d_sb.rearrange("p (o c) -> p o c", o=1).to_broadcast([P, NCAND, C])
        a_b = a_sb.rearrange("p (o c) -> p o c", o=1).to_broadcast([P, NCAND, C])

        if not fast_scatter:
            # general path: gather y back, then z0 / tau sums in one shot
            nc.sync.dma_start(out=y2, in_=yt_dram.rearrange("(p c) -> p c", p=P))
            nc.vector.tensor_tensor(out=z0, in0=y2, in1=ci_sb, op=Alu.mult)
            nc.vector.tensor_scalar(out=z0, in0=z0, scalar1=-1.0, scalar2=None,
                                    op0=Alu.add)
            nc.vector.tensor_scalar(out=tmp, in0=z0, scalar1=EPS, scalar2=-EPS,
                                    op0=Alu.min, op1=Alu.max)
            if not no_pad:
                nc.vector.tensor_tensor(out=tmp, in0=tmp, in1=a_sb, op=Alu.mult)
            nc.vector.tensor_reduce(out=red, in_=tmp,
                                    axis=mybir.AxisListType.X, op=Alu.add)
        # taun = clip(S(0), +-DELTA*n): partials in red were accumulated
        # per block during the MLP; finish with the partition reduce.
        t0ps = pst.tile([P, NCAND], f32, tag="hps", name="t0ps")
        nc.tensor.matmul(t0ps[:, 0:1], lhsT=onesmat, rhs=red, start=True,
                         stop=True)
        dn = DELTA * (float(nrows) if no_pad else 1.0)
        nc.vector.tensor_scalar(out=taun, in0=t0ps[:, 0:1], scalar1=dn,
                                scalar2=-dn, op0=Alu.min, op1=Alu.max)

        nc.vector.memset(los[0], -W0 / 2.0)
        lo = los[0]
        width = W0
        for r_i in range(N_ROUNDS):
            step = width / (NCAND + 1)
            # candidate lambdas: lo + j*step, j = 1..NCAND
            nc.vector.tensor_scalar(out=lamc, in0=iota, scalar1=step,
                                    scalar2=lo[:, 0:1], op0=Alu.mult,
                                    op1=Alu.add)
            lam_b = lamc.rearrange("p (o j) -> p j o", o=1).to_broadcast(
                [P, NCAND, C])
            nc.vector.tensor_tensor(out=tmp3, in0=d_b, in1=lam_b, op=Alu.mult)
            nc.vector.tensor_tensor(out=tmp3, in0=tmp3, in1=z0b, op=Alu.add)
            nc.vector.tensor_scalar(out=tmp3, in0=tmp3, scalar1=EPS,
                                    scalar2=-EPS, op0=Alu.min, op1=Alu.max)
            if not no_pad:
                nc.vector.tensor_tensor(out=tmp3, in0=tmp3, in1=a_b,
                                        op=Alu.mult)
            nc.vector.tensor_reduce(out=red7, in_=tmp3,
                                    axis=mybir.AxisListType.X, op=Alu.add)
            hps = pst.tile([P, NCAND], f32, tag="hps", name=f"hps{r_i}")
            nc.tensor.matmul(hps, lhsT=onesmat, rhs=red7, start=True,
                             stop=True)
            nc.vector.tensor_scalar(out=predl, in0=hps, scalar1=taun[:, 0:1],
                                    scalar2=None, op0=Alu.is_le)
            nc.vector.tensor_reduce(out=cnt, in_=predl,
                                    axis=mybir.AxisListType.X, op=Alu.add)
            lo_next = los[(r_i + 1) % 2]
            nc.vector.tensor_scalar(out=lo_next, in0=cnt, scalar1=step,
                                    scalar2=lo[:, 0:1], op0=Alu.mult,
                                    op1=Alu.add)
            lo = lo_next
            width = step  # interval shrinks to one candidate slot

        # final: lam = lo + width/2 ; out = (clip(z0 + lam*d) + 1) * c
        lamf = proj.tile([P, 1], f32, tag="lamf")
        nc.vector.tensor_scalar(out=lamf, in0=lo, scalar1=width / 2.0,
                                scalar2=None, op0=Alu.add)
        nc.vector.tensor_scalar(out=tmp, in0=d_sb, scalar1=lamf[:, 0:1],
                                scalar2=None, op0=Alu.mult)
        nc.vector.tensor_tensor(out=tmp, in0=tmp, in1=z0, op=Alu.add)
        nc.vector.tensor_scalar(out=tmp, in0=tmp, scalar1=EPS, scalar2=-EPS,
                                op0=Alu.min, op1=Alu.max)
        nc.vector.tensor_scalar(out=tmp, in0=tmp, scalar1=1.0, scalar2=None,
                                op0=Alu.add)
        nc.vector.tensor_tensor(out=tmp, in0=tmp, in1=c_sb, op=Alu.mult)
        nc.sync.dma_start(out=out2d, in_=tmp)

    nc.compile()
    return nc


def kernel(**inputs):
    global LAST_RESULT
    x = np.ascontiguousarray(np.asarray(inputs["x"], dtype=np.float32))
    W1 = np.ascontiguousarray(np.asarray(inputs["W1"], dtype=np.float32))
    b1 = np.ascontiguousarray(np.asarray(inputs["b1"], dtype=np.float32))
    W2 = np.ascontiguousarray(np.asarray(inputs["W2"], dtype=np.float32))
    b2 = np.ascontiguousarray(np.asarray(inputs["b2"], dtype=np.float32))
    Wf = np.ascontiguousarray(np.asarray(inputs["Wf"], dtype=np.float32))
    bf = float(np.asarray(inputs["bf"], dtype=np.float32).reshape(-1)[0])
    c = np.ascontiguousarray(np.asarray(inputs["constraint_constant"], dtype=np.float32))
    gm = np.asarray(inputs["group_mask"], dtype=np.float32)

    N, D = x.shape
    H1 = W1.shape[1]
    H2 = W2.shape[1]
    G = gm.shape[0]
    assert G == 8, "this kernel shards one quantile group per core"
    assert D % P == 0 and H1 % P == 0 and H2 % P == 0 and Wf.shape[1] == 1

    g = np.argmax(gm, axis=0)
    sizes = np.bincount(g, minlength=G)
    R = int(-(-sizes.max() // BLK) * BLK)   # per-core padded rows
    C = R // P
    uniform = bool((sizes == sizes[0]).all() and sizes[0] == R)

    order = np.argsort(g, kind="stable")
    bounds = np.zeros(G + 1, np.int64)
    np.cumsum(sizes, out=bounds[1:])

    def to2d(vec):
        # local row r -> (p = r // C, col = r % C)
        return np.ascontiguousarray(vec.reshape(P, C))

    in_maps = []
    rows_per_core = []
    for j in range(G):
        rows = order[bounds[j]:bounds[j + 1]]
        nrows = rows.shape[0]
        rows_per_core.append(rows)

        xtj = np.zeros((D, R), np.float32)
        xtj[:, :nrows] = x[rows].T   # row gather (contiguous) then T-assign

        cj = np.ones(R, np.float32)
        cj[:nrows] = c[rows]
        cij = 1.0 / cj
        dj = np.zeros(R, np.float32)
        dj[:nrows] = cij[:nrows] * cij[:nrows]
        aj = np.zeros(R, np.float32)
        aj[:nrows] = 1.0 / nrows

        in_maps.append(dict(
            xt=xtj, w1=W1, w2=W2, wf=Wf.reshape(-1), b1=b1, b2=b2,
            c2d=to2d(cj), ci2d=to2d(cij), d2d=to2d(dj), a2d=to2d(aj),
        ))

    nrows_build = R if uniform else -1   # -1 -> general padded path
    key = (D, H1, H2, R, nrows_build, float(bf))
    nc = _PROGRAM_CACHE.get(key)
    if nc is None:
        nc = _build_program(D, H1, H2, R, nrows_build, float(bf))
        _PROGRAM_CACHE[key] = nc

    from concourse.bass_utils import run_bass_kernel_spmd
    trace = bool(int(os.environ.get("KERNEL_PROFILE", "0")))
    res = run_bass_kernel_spmd(nc, in_maps, list(range(G)), trace=trace)
    LAST_RESULT = res

    out = np.empty((N, 1), np.float32)
    for j in range(G):
        y2d = res.results[j]["out2d"]          # [128, C], row r = p*C + col
        yvec = y2d.reshape(-1)
        out[rows_per_core[j], 0] = yvec[:rows_per_core[j].shape[0]]
    return out


# revision 17
# speedup vs baseline: 1.0239x; 1.0219x over previous
"""ConstrainedMLP Trainium2 kernel.

Strategy
--------
Rows are sharded BY GROUP (the quantile groups of `group_mask`): there are
exactly 8 groups and 8 cores, so each core owns one group and the
constrained-projection step becomes fully core-local (no collectives).

Per core:
  MLP   : h1 = relu(x@W1+b1), h2 = relu(h1@W2+b2), y_u = h2@Wf+bf
          computed in "transposed activation" layout (h1T/h2T = [feat, rows])
          with fp32r matmuls so every matmul has a 512-wide moving operand.
  Proj  : the reference's 300-iteration Dykstra projection converges to the
          exact QP solution  z* = clip(z0 + lam/w, +-EPS)  with a single
          scalar lam per group chosen so the group mean of z* hits
          clip(mean(clip(z0)), +-DELTA).  lam is found with a 7-round
          8-ary search (7 candidates per round, 3 bits/round); all search
          state is replicated across the 128 partitions so each round is a
          short DVE chain plus one all-ones matmul for the cross-partition
          reduction.  (Validated vs the 300-iter reference: ~2e-6.)

Self-contained: only numpy + the concourse/bass runtime installed in the
environment. No files are read.
"""

import os
import numpy as np

EPS = 0.15
DELTA = 0.05
W0 = 16.0        # initial lambda interval: [-8, 8] (lam* <= 2.7, 3x margin)
N_ROUNDS = 6     # 8-ary search rounds -> final width 16/8^6 ~ 6.1e-5
NCAND = 7        # candidates per round
P = 128          # SBUF partitions
BLK = 512        # row block (moving-operand width)

_PROGRAM_CACHE = {}
LAST_RESULT = None  # test harness introspection (exec_time etc.)


def _build_program(D, H1, H2, R, nrows, bf_val):
    import concourse.bass as bass
    import concourse.tile as tile
    from concourse import bacc, mybir
    from contextlib import ExitStack

    f32 = mybir.dt.float32
    f32r = mybir.dt.float32r
    Alu = mybir.AluOpType
    Act = mybir.ActivationFunctionType

    KD = D // P       # contraction chunks for layer 1
    K1 = H1 // P      # h1 feature chunks
    K2 = H2 // P      # h2 feature chunks
    NB = R // BLK     # row blocks
    C = R // P        # columns of the [128, C] projection layout
    fast_scatter = (BLK % C == 0)
    PPB = BLK // C if fast_scatter else 0  # partitions covered per row block
    no_pad = (nrows == R)

    nc = bacc.Bacc("TRN2", target_bir_lowering=False, debug=False, num_devices=8)

    xt = nc.dram_tensor("xt", [D, R], f32, kind="ExternalInput").ap()
    w1 = nc.dram_tensor("w1", [D, H1], f32, kind="ExternalInput").ap()
    w2 = nc.dram_tensor("w2", [H1, H2], f32, kind="ExternalInput").ap()
    wf = nc.dram_tensor("wf", [H2], f32, kind="ExternalInput").ap()
    b1 = nc.dram_tensor("b1", [H1], f32, kind="ExternalInput").ap()
    b2 = nc.dram_tensor("b2", [H2], f32, kind="ExternalInput").ap()
    c2d = nc.dram_tensor("c2d", [P, C], f32, kind="ExternalInput").ap()
    ci2d = nc.dram_tensor("ci2d", [P, C], f32, kind="ExternalInput").ap()
    d2d = nc.dram_tensor("d2d", [P, C], f32, kind="ExternalInput").ap()
    a2d = nc.dram_tensor("a2d", [P, C], f32, kind="ExternalInput").ap()
    out2d = nc.dram_tensor("out2d", [P, C], f32, kind="ExternalOutput").ap()

    with tile.TileContext(nc) as tc, ExitStack() as ctx:
        consts = ctx.enter_context(tc.tile_pool(name="consts", bufs=1))
        xpool = ctx.enter_context(tc.tile_pool(name="xp", bufs=6))
        h1pool = ctx.enter_context(tc.tile_pool(name="h1p", bufs=3))
        h2pool = ctx.enter_context(tc.tile_pool(name="h2p", bufs=3))
        ytpool = ctx.enter_context(tc.tile_pool(name="ytp", bufs=2))
        ps1 = ctx.enter_context(tc.tile_pool(name="ps1", bufs=3, space="PSUM"))
        ps2 = ctx.enter_context(tc.tile_pool(name="ps2", bufs=3, space="PSUM"))
        psy = ctx.enter_context(tc.tile_pool(name="psy", bufs=1, space="PSUM"))
        pst = ctx.enter_context(tc.tile_pool(name="pst", bufs=1, space="PSUM"))
        proj = ctx.enter_context(tc.tile_pool(name="proj", bufs=1))
        if not fast_scatter:
            dramp = ctx.enter_context(tc.tile_pool(name="dramp", bufs=1,
                                                   space="DRAM"))
            yt_dram = dramp.tile([R], f32, name="yt_dram")

        onesmat = consts.tile([P, P], f32, tag="onesmat")
        nc.vector.memset(onesmat, 1.0)
        iota = consts.tile([P, NCAND], f32, tag="iota")
        for j in range(NCAND):
            nc.vector.memset(iota[:, j:j + 1], float(j + 1))
        # PE warmup: dummy matmuls with no data deps run during the startup
        # DMA window so the HAM clock-gate is already at 2.4GHz when the
        # first real matmul issues (saves the ~3.4us cold-clock ramp).
        warm_junk = proj.tile([P, 1], f32, tag="warm_junk")
        for wi in range(10):
            wps = ps1.tile([P, BLK], f32, tag="ps1", name=f"warm{wi}")
            nc.tensor.matmul(wps[:, 0:P], lhsT=onesmat, rhs=onesmat,
                             start=True, stop=True)
            if wi == 9:
                nc.scalar.activation(out=warm_junk, in_=wps[:, 0:1],
                                     func=Act.Copy, bias=0.0, scale=1.0)

        # ---- resident weights / constants ----
        # Single ordered HWDGE stream arranged so the first L1 matmuls can
        # start as early as possible: w1[0], block-0 x tiles, w1[1:], b1,
        # then layer-2 / projection constants, then the b>=1 x stream.
        w1sb = [consts.tile([P, H1], f32r, tag=f"w1_{k}", name=f"w1sb{k}")
                for k in range(KD)]
        xts0 = [xpool.tile([P, BLK], f32r, tag=f"x{k}", name=f"x0_{k}")
                for k in range(KD)]
        for k in range(KD):
            nc.sync.dma_start(out=w1sb[k], in_=w1[k * P:(k + 1) * P, :].bitcast(f32r))
            nc.sync.dma_start(out=xts0[k], in_=xt[k * P:(k + 1) * P, 0:BLK].bitcast(f32r))
        b1sb = consts.tile([P, K1], f32, tag="b1")
        nc.sync.dma_start(out=b1sb, in_=b1.rearrange("(m p) -> p m", p=P))

        w2sb = []
        for k in range(K1):
            t = consts.tile([P, H2], f32r, tag=f"w2_{k}")
            nc.sync.dma_start(out=t, in_=w2[k * P:(k + 1) * P, :].bitcast(f32r))
            w2sb.append(t)
        wfsb = consts.tile([P, K2], f32r, tag="wf")
        nc.sync.dma_start(out=wfsb, in_=wf.rearrange("(k p) -> p k", p=P).bitcast(f32r))
        b2sb = consts.tile([P, K2], f32, tag="b2")
        nc.sync.dma_start(out=b2sb, in_=b2.rearrange("(m p) -> p m", p=P))

        ci_sb = consts.tile([P, C], f32, tag="ci_sb")
        nc.gpsimd.dma_start(out=ci_sb, in_=ci2d)
        c_sb = consts.tile([P, C], f32, tag="c_sb")
        nc.gpsimd.dma_start(out=c_sb, in_=c2d)
        d_sb = consts.tile([P, C], f32, tag="d_sb")
        nc.gpsimd.dma_start(out=d_sb, in_=d2d)
        a_sb = consts.tile([P, C], f32, tag="a_sb")
        nc.gpsimd.dma_start(out=a_sb, in_=a2d)

        y2 = consts.tile([P, C], f32, tag="y2")
        z0 = proj.tile([P, C], f32, tag="z0")
        tmp = proj.tile([P, C], f32, tag="tmp")
        red = proj.tile([P, 1], f32, tag="red")

        # ---- MLP over row blocks ----
        for b in range(NB):
            cols = slice(b * BLK, (b + 1) * BLK)
            if b == 0:
                xts = xts0
            else:
                xts = []
                for k in range(KD):
                    t = xpool.tile([P, BLK], f32r, tag=f"x{k}", name=f"x{b}_{k}")
                    nc.sync.dma_start(out=t, in_=xt[k * P:(k + 1) * P, cols].bitcast(f32r))
                    xts.append(t)

            h1t = h1pool.tile([P, K1, BLK], f32r, tag="h1t")
            for m in range(K1):
                pt = ps1.tile([P, BLK], f32, tag="ps1")
                for k in range(KD):
                    nc.tensor.matmul(
                        pt,
                        lhsT=w1sb[k][:, m * P:(m + 1) * P],
                        rhs=xts[k][:, :],
                        start=(k == 0),
                        stop=(k == KD - 1),
                    )
                nc.scalar.activation(
                    out=h1t[:, m, :], in_=pt, func=Act.Relu,
                    bias=b1sb[:, m:m + 1], scale=1.0,
                )

            h2t = h2pool.tile([P, K2, BLK], f32r, tag="h2t")
            for m in range(K2):
                pt = ps2.tile([P, BLK], f32, tag="ps2")
                for k in range(K1):
                    nc.tensor.matmul(
                        pt,
                        lhsT=w2sb[k][:, m * P:(m + 1) * P],
                        rhs=h1t[:, k, :],
                        start=(k == 0),
                        stop=(k == K1 - 1),
                    )
                # relu+bias on the vector engine to keep ScalarE headroom
                nc.vector.tensor_scalar(
                    out=h2t[:, m, :], in0=pt, scalar1=b2sb[:, m:m + 1],
                    scalar2=0.0, op0=Alu.add, op1=Alu.max,
                )

            # final layer: yT[1, BLK] = sum_k wf_k.T @ h2t_k  (fp32r, N=512)
            pty = psy.tile([1, BLK], f32, tag="psy")
            for k in range(K2):
                nc.tensor.matmul(
                    pty,
                    lhsT=wfsb[:, k:k + 1],
                    rhs=h2t[:, k, :],
                    start=(k == 0),
                    stop=(k == K2 - 1),
                )
            ytb = ytpool.tile([1, BLK], f32, tag="ytb")
            nc.scalar.activation(out=ytb, in_=pty, func=Act.Copy,
                                 bias=float(bf_val), scale=1.0)
            # scatter rows b*BLK..(b+1)*BLK into y2[p, c] with r = p*C + c
            if fast_scatter:
                nc.sync.dma_start(out=y2[b * PPB:(b + 1) * PPB, :],
                                  in_=ytb[0:1, :])
            else:
                nc.sync.dma_start(out=yt_dram[b * BLK:(b + 1) * BLK],
                                  in_=ytb[0:1, :])
            # every 4 blocks (engine partition offsets must be 32-aligned):
            # z0 = y*cinv - 1 and tau partial sums on the vector engine,
            # which is idle during the MLP — hides the projection preamble.
            if fast_scatter and (b + 1) * PPB % 32 == 0:
                pp = slice((b + 1) * PPB - 32, (b + 1) * PPB)
                nc.vector.tensor_tensor(out=z0[pp, :], in0=y2[pp, :],
                                        in1=ci_sb[pp, :], op=Alu.mult)
                nc.vector.tensor_scalar(out=z0[pp, :], in0=z0[pp, :],
                                        scalar1=-1.0, scalar2=None, op0=Alu.add)
                nc.vector.tensor_scalar(out=tmp[pp, :], in0=z0[pp, :],
                                        scalar1=EPS, scalar2=-EPS,
                                        op0=Alu.min, op1=Alu.max)
                if not no_pad:
                    nc.vector.tensor_tensor(out=tmp[pp, :], in0=tmp[pp, :],
                                            in1=a_sb[pp, :], op=Alu.mult)
                nc.vector.tensor_reduce(out=red[pp, :], in_=tmp[pp, :],
                                        axis=mybir.AxisListType.X, op=Alu.add)

        # ---- projection ----
        # z* = clip(z0 + lam*d, +-EPS) with scalar lam s.t.
        #   S(lam) := sum_r m_r clip(z0_r + lam*d_r)  ==  taun,
        #   taun = clip(S(0), +-DELTA*n)   (m absorbed in a2d when padded;
        #                                   m == 1 and plain sums when not)
        # 7-round 8-ary search over lam; state replicated across partitions,
        # each round = one DVE chain + one all-ones matmul (partition reduce).
        tmp3 = proj.tile([P, NCAND, C], f32, tag="tmp3")
        red7 = proj.tile([P, NCAND], f32, tag="red7")
        lamc = proj.tile([P, NCAND], f32, tag="lamc")
        predl = proj.tile([P, NCAND], f32, tag="predl")
        cnt = proj.tile([P, 1], f32, tag="cnt")
        taun = proj.tile([P, 1], f32, tag="taun")
        los = [proj.tile([P, 1], f32, tag=f"lo{i}", name=f"lo{i}")
               for i in range(2)]

        z0b = z0.rearrange("p (o c) -> p o c", o=1).to_broadcast([P, NCAND, C])
        d_b = d_sb.rearrange("p (o c) -> p o c", o=1).to_broadcast([P, NCAND, C])
        a_b = a_sb.rearrange("p (o c) -> p o c", o=1).to_broadcast([P, NCAND, C])

        if not fast_scatter:
            # general path: gather y back, then z0 / tau sums in one shot
            nc.sync.dma_start(out=y2, in_=yt_dram.rearrange("(p c) -> p c", p=P))
            nc.vector.tensor_tensor(out=z0, in0=y2, in1=ci_sb, op=Alu.mult)
            nc.vector.tensor_scalar(out=z0, in0=z0, scalar1=-1.0, scalar2=None,
                                    op0=Alu.add)
            nc.vector.tensor_scalar(out=tmp, in0=z0, scalar1=EPS, scalar2=-EPS,
                                    op0=Alu.min, op1=Alu.max)
            if not no_pad:
                nc.vector.tensor_tensor(out=tmp, in0=tmp, in1=a_sb, op=Alu.mult)
            nc.vector.tensor_reduce(out=red, in_=tmp,
                                    axis=mybir.AxisListType.X, op=Alu.add)
        # taun = clip(S(0), +-DELTA*n): partials in red were accumulated
        # per block during the MLP; finish with the partition reduce.
        t0ps = pst.tile([P, NCAND], f32, tag="hps", name="t0ps")
        nc.tensor.matmul(t0ps[:, 0:1], lhsT=onesmat, rhs=red, start=True,
                         stop=True)
        dn = DELTA * (float(nrows) if no_pad else 1.0)
        nc.vector.tensor_scalar(out=taun, in0=t0ps[:, 0:1], scalar1=dn,
                                scalar2=-dn, op0=Alu.min, op1=Alu.max)

        nc.vector.memset(los[0], -W0 / 2.0)
        lo = los[0]
        width = W0
        for r_i in range(N_ROUNDS):
            step = width / (NCAND + 1)
            # candidate lambdas: lo + j*step, j = 1..NCAND
            nc.vector.tensor_scalar(out=lamc, in0=iota, scalar1=step,
                                    scalar2=lo[:, 0:1], op0=Alu.mult,
                                    op1=Alu.add)
            lam_b = lamc.rearrange("p (o j) -> p j o", o=1).to_broadcast(
                [P, NCAND, C])
            nc.vector.tensor_tensor(out=tmp3, in0=d_b, in1=lam_b, op=Alu.mult)
            nc.vector.tensor_tensor(out=tmp3, in0=tmp3, in1=z0b, op=Alu.add)
            nc.vector.tensor_scalar(out=tmp3, in0=tmp3, scalar1=EPS,
                                    scalar2=-EPS, op0=Alu.min, op1=Alu.max)
            if not no_pad:
                nc.vector.tensor_tensor(out=tmp3, in0=tmp3, in1=a_b,
                                        op=Alu.mult)
            nc.vector.tensor_reduce(out=red7, in_=tmp3,
                                    axis=mybir.AxisListType.X, op=Alu.add)
            hps = pst.tile([P, NCAND], f32, tag="hps", name=f"hps{r_i}")
            nc.tensor.matmul(hps, lhsT=onesmat, rhs=red7, start=True,
                             stop=True)
            nc.vector.tensor_scalar(out=predl, in0=hps, scalar1=taun[:, 0:1],
                                    scalar2=None, op0=Alu.is_le)
            nc.vector.tensor_reduce(out=cnt, in_=predl,
                                    axis=mybir.AxisListType.X, op=Alu.add)
            lo_next = los[(r_i + 1) % 2]
            nc.vector.tensor_scalar(out=lo_next, in0=cnt, scalar1=step,
                                    scalar2=lo[:, 0:1], op0=Alu.mult,
                                    op1=Alu.add)
            lo = lo_next
            width = step  # interval shrinks to one candidate slot

        # final: lam = lo + width/2 ; out = (clip(z0 + lam*d) + 1) * c
        lamf = proj.tile([P, 1], f32, tag="lamf")
        nc.vector.tensor_scalar(out=lamf, in0=lo, scalar1=width / 2.0,
                                scalar2=None, op0=Alu.add)
        nc.vector.tensor_scalar(out=tmp, in0=d_sb, scalar1=lamf[:, 0:1],
                                scalar2=None, op0=Alu.mult)
        nc.vector.tensor_tensor(out=tmp, in0=tmp, in1=z0, op=Alu.add)
        nc.vector.tensor_scalar(out=tmp, in0=tmp, scalar1=EPS, scalar2=-EPS,
                                op0=Alu.min, op1=Alu.max)
        nc.vector.tensor_scalar(out=tmp, in0=tmp, scalar1=1.0, scalar2=None,
                                op0=Alu.add)
        nc.vector.tensor_tensor(out=tmp, in0=tmp, in1=c_sb, op=Alu.mult)
        nc.sync.dma_start(out=out2d, in_=tmp)

    nc.compile()
    return nc


def kernel(**inputs):
    global LAST_RESULT
    x = np.ascontiguousarray(np.asarray(inputs["x"], dtype=np.float32))
    W1 = np.ascontiguousarray(np.asarray(inputs["W1"], dtype=np.float32))
    b1 = np.ascontiguousarray(np.asarray(inputs["b1"], dtype=np.float32))
    W2 = np.ascontiguousarray(np.asarray(inputs["W2"], dtype=np.float32))
    b2 = np.ascontiguousarray(np.asarray(inputs["b2"], dtype=np.float32))
    Wf = np.ascontiguousarray(np.asarray(inputs["Wf"], dtype=np.float32))
    bf = float(np.asarray(inputs["bf"], dtype=np.float32).reshape(-1)[0])
    c = np.ascontiguousarray(np.asarray(inputs["constraint_constant"], dtype=np.float32))
    gm = np.asarray(inputs["group_mask"], dtype=np.float32)

    N, D = x.shape
    H1 = W1.shape[1]
    H2 = W2.shape[1]
    G = gm.shape[0]
    assert G == 8, "this kernel shards one quantile group per core"
    assert D % P == 0 and H1 % P == 0 and H2 % P == 0 and Wf.shape[1] == 1

    g = np.argmax(gm, axis=0)
    sizes = np.bincount(g, minlength=G)
    R = int(-(-sizes.max() // BLK) * BLK)   # per-core padded rows
    C = R // P
    uniform = bool((sizes == sizes[0]).all() and sizes[0] == R)

    order = np.argsort(g, kind="stable")
    bounds = np.zeros(G + 1, np.int64)
    np.cumsum(sizes, out=bounds[1:])

    def to2d(vec):
        # local row r -> (p = r // C, col = r % C)
        return np.ascontiguousarray(vec.reshape(P, C))

    in_maps = []
    rows_per_core = []
    for j in range(G):
        rows = order[bounds[j]:bounds[j + 1]]
        nrows = rows.shape[0]
        rows_per_core.append(rows)

        xtj = np.zeros((D, R), np.float32)
        xtj[:, :nrows] = x[rows].T   # row gather (contiguous) then T-assign

        cj = np.ones(R, np.float32)
        cj[:nrows] = c[rows]
        cij = 1.0 / cj
        dj = np.zeros(R, np.float32)
        dj[:nrows] = cij[:nrows] * cij[:nrows]
        aj = np.zeros(R, np.float32)
        aj[:nrows] = 1.0 / nrows

        in_maps.append(dict(
            xt=xtj, w1=W1, w2=W2, wf=Wf.reshape(-1), b1=b1, b2=b2,
            c2d=to2d(cj), ci2d=to2d(cij), d2d=to2d(dj), a2d=to2d(aj),
        ))

    nrows_build = R if uniform else -1   # -1 -> general padded path
    key = (D, H1, H2, R, nrows_build, float(bf))
    nc = _PROGRAM_CACHE.get(key)
    if nc is None:
        nc = _build_program(D, H1, H2, R, nrows_build, float(bf))
        _PROGRAM_CACHE[key] = nc

    from concourse.bass_utils import run_bass_kernel_spmd
    trace = bool(int(os.environ.get("KERNEL_PROFILE", "0")))
    res = run_bass_kernel_spmd(nc, in_maps, list(range(G)), trace=trace)
    LAST_RESULT = res

    out = np.empty((N, 1), np.float32)
    for j in range(G):
        y2d = res.results[j]["out2d"]          # [128, C], row r = p*C + col
        yvec = y2d.reshape(-1)
        out[rows_per_core[j], 0] = yvec[:rows_per_core[j].shape[0]]
    return out


# revision 18
# speedup vs baseline: 1.0322x; 1.0081x over previous
"""ConstrainedMLP Trainium2 kernel.

Strategy
--------
Rows are sharded BY GROUP (the quantile groups of `group_mask`): there are
exactly 8 groups and 8 cores, so each core owns one group and the
constrained-projection step becomes fully core-local (no collectives).

Per core:
  MLP   : h1 = relu(x@W1+b1), h2 = relu(h1@W2+b2), y_u = h2@Wf+bf
          computed in "transposed activation" layout (h1T/h2T = [feat, rows])
          with fp32r matmuls so every matmul has a 512-wide moving operand.
  Proj  : the reference's 300-iteration Dykstra projection converges to the
          exact QP solution  z* = clip(z0 + lam/w, +-EPS)  with a single
          scalar lam per group chosen so the group mean of z* hits
          clip(mean(clip(z0)), +-DELTA).  lam is found with a 7-round
          8-ary search (7 candidates per round, 3 bits/round); all search
          state is replicated across the 128 partitions so each round is a
          short DVE chain plus one all-ones matmul for the cross-partition
          reduction.  (Validated vs the 300-iter reference: ~2e-6.)

Self-contained: only numpy + the concourse/bass runtime installed in the
environment. No files are read.
"""

import os
import numpy as np

EPS = 0.15
DELTA = 0.05
W0 = 16.0        # initial lambda interval: [-8, 8] (lam* <= 2.7, 3x margin)
N_ROUNDS = 6     # 8-ary search rounds -> final width 16/8^6 ~ 6.1e-5
NCAND = 7        # candidates per round
P = 128          # SBUF partitions
BLK = 512        # row block (moving-operand width)

_PROGRAM_CACHE = {}
LAST_RESULT = None  # test harness introspection (exec_time etc.)


def _build_program(D, H1, H2, R, nrows, bf_val):
    import concourse.bass as bass
    import concourse.tile as tile
    from concourse import bacc, mybir
    from contextlib import ExitStack

    f32 = mybir.dt.float32
    f32r = mybir.dt.float32r
    Alu = mybir.AluOpType
    Act = mybir.ActivationFunctionType

    KD = D // P       # contraction chunks for layer 1
    K1 = H1 // P      # h1 feature chunks
    K2 = H2 // P      # h2 feature chunks
    NB = R // BLK     # row blocks
    C = R // P        # columns of the [128, C] projection layout
    fast_scatter = (BLK % C == 0)
    PPB = BLK // C if fast_scatter else 0  # partitions covered per row block
    no_pad = (nrows == R)

    nc = bacc.Bacc("TRN2", target_bir_lowering=False, debug=False, num_devices=8)

    xt = nc.dram_tensor("xt", [D, R], f32, kind="ExternalInput").ap()
    w1 = nc.dram_tensor("w1", [D, H1], f32, kind="ExternalInput").ap()
    w2 = nc.dram_tensor("w2", [H1, H2], f32, kind="ExternalInput").ap()
    wf = nc.dram_tensor("wf", [H2], f32, kind="ExternalInput").ap()
    b1 = nc.dram_tensor("b1", [H1], f32, kind="ExternalInput").ap()
    b2 = nc.dram_tensor("b2", [H2], f32, kind="ExternalInput").ap()
    c2d = nc.dram_tensor("c2d", [P, C], f32, kind="ExternalInput").ap()
    ci2d = nc.dram_tensor("ci2d", [P, C], f32, kind="ExternalInput").ap()
    d2d = nc.dram_tensor("d2d", [P, C], f32, kind="ExternalInput").ap()
    a2d = nc.dram_tensor("a2d", [P, C], f32, kind="ExternalInput").ap()
    out2d = nc.dram_tensor("out2d", [P, C], f32, kind="ExternalOutput").ap()

    with tile.TileContext(nc) as tc, ExitStack() as ctx:
        consts = ctx.enter_context(tc.tile_pool(name="consts", bufs=1))
        xpool = ctx.enter_context(tc.tile_pool(name="xp", bufs=6))
        h1pool = ctx.enter_context(tc.tile_pool(name="h1p", bufs=3))
        h2pool = ctx.enter_context(tc.tile_pool(name="h2p", bufs=3))
        ytpool = ctx.enter_context(tc.tile_pool(name="ytp", bufs=2))
        ps1 = ctx.enter_context(tc.tile_pool(name="ps1", bufs=3, space="PSUM"))
        ps2 = ctx.enter_context(tc.tile_pool(name="ps2", bufs=3, space="PSUM"))
        psy = ctx.enter_context(tc.tile_pool(name="psy", bufs=1, space="PSUM"))
        pst = ctx.enter_context(tc.tile_pool(name="pst", bufs=1, space="PSUM"))
        proj = ctx.enter_context(tc.tile_pool(name="proj", bufs=1))
        if not fast_scatter:
            dramp = ctx.enter_context(tc.tile_pool(name="dramp", bufs=1,
                                                   space="DRAM"))
            yt_dram = dramp.tile([R], f32, name="yt_dram")

        onesmat = consts.tile([P, P], f32, tag="onesmat")
        nc.vector.memset(onesmat, 1.0)
        iota = consts.tile([P, NCAND], f32, tag="iota")
        for j in range(NCAND):
            nc.vector.memset(iota[:, j:j + 1], float(j + 1))
        # PE warmup: dummy matmuls with no data deps run during the startup
        # DMA window so the HAM clock-gate is already at 2.4GHz when the
        # first real matmul issues (saves the ~3.4us cold-clock ramp).
        warm_junk = proj.tile([P, 1], f32, tag="warm_junk")
        for wi in range(10):
            wps = ps1.tile([P, BLK], f32, tag="ps1", name=f"warm{wi}")
            nc.tensor.matmul(wps[:, 0:P], lhsT=onesmat, rhs=onesmat,
                             start=True, stop=True)
            if wi == 9:
                nc.scalar.activation(out=warm_junk, in_=wps[:, 0:1],
                                     func=Act.Copy, bias=0.0, scale=1.0)

        # ---- resident weights / constants ----
        # Single ordered HWDGE stream arranged so the first L1 matmuls can
        # start as early as possible: w1[0], block-0 x tiles, w1[1:], b1,
        # then layer-2 / projection constants, then the b>=1 x stream.
        w1sb = [consts.tile([P, H1], f32r, tag=f"w1_{k}", name=f"w1sb{k}")
                for k in range(KD)]
        xts0 = [xpool.tile([P, BLK], f32r, tag=f"x{k}", name=f"x0_{k}")
                for k in range(KD)]
        b1sb = consts.tile([P, K1], f32, tag="b1")
        nc.sync.dma_start(out=b1sb, in_=b1.rearrange("(m p) -> p m", p=P))
        # first column-half of each w1 chunk + block-0 x first: the first
        # four m-groups can start ~3us earlier than waiting for full w1
        H1h = H1 // 2
        for k in range(KD):
            nc.sync.dma_start(out=w1sb[k][:, 0:H1h],
                              in_=w1[k * P:(k + 1) * P, 0:H1h].bitcast(f32r))
            nc.sync.dma_start(out=xts0[k], in_=xt[k * P:(k + 1) * P, 0:BLK].bitcast(f32r))
        for k in range(KD):
            nc.sync.dma_start(out=w1sb[k][:, H1h:H1],
                              in_=w1[k * P:(k + 1) * P, H1h:H1].bitcast(f32r))

        w2sb = []
        for k in range(K1):
            t = consts.tile([P, H2], f32r, tag=f"w2_{k}")
            nc.sync.dma_start(out=t, in_=w2[k * P:(k + 1) * P, :].bitcast(f32r))
            w2sb.append(t)
        wfsb = consts.tile([P, K2], f32r, tag="wf")
        nc.sync.dma_start(out=wfsb, in_=wf.rearrange("(k p) -> p k", p=P).bitcast(f32r))
        b2sb = consts.tile([P, K2], f32, tag="b2")
        nc.sync.dma_start(out=b2sb, in_=b2.rearrange("(m p) -> p m", p=P))

        ci_sb = consts.tile([P, C], f32, tag="ci_sb")
        nc.gpsimd.dma_start(out=ci_sb, in_=ci2d)
        c_sb = consts.tile([P, C], f32, tag="c_sb")
        nc.gpsimd.dma_start(out=c_sb, in_=c2d)
        d_sb = consts.tile([P, C], f32, tag="d_sb")
        nc.gpsimd.dma_start(out=d_sb, in_=d2d)
        a_sb = consts.tile([P, C], f32, tag="a_sb")
        nc.gpsimd.dma_start(out=a_sb, in_=a2d)

        y2 = consts.tile([P, C], f32, tag="y2")
        z0 = proj.tile([P, C], f32, tag="z0")
        tmp = proj.tile([P, C], f32, tag="tmp")
        red = proj.tile([P, 1], f32, tag="red")

        # ---- MLP over row blocks ----
        for b in range(NB):
            cols = slice(b * BLK, (b + 1) * BLK)
            if b == 0:
                xts = xts0
            else:
                xts = []
                for k in range(KD):
                    t = xpool.tile([P, BLK], f32r, tag=f"x{k}", name=f"x{b}_{k}")
                    nc.sync.dma_start(out=t, in_=xt[k * P:(k + 1) * P, cols].bitcast(f32r))
                    xts.append(t)

            h1t = h1pool.tile([P, K1, BLK], f32r, tag="h1t")
            for m in range(K1):
                pt = ps1.tile([P, BLK], f32, tag="ps1")
                for k in range(KD):
                    nc.tensor.matmul(
                        pt,
                        lhsT=w1sb[k][:, m * P:(m + 1) * P],
                        rhs=xts[k][:, :],
                        start=(k == 0),
                        stop=(k == KD - 1),
                    )
                nc.scalar.activation(
                    out=h1t[:, m, :], in_=pt, func=Act.Relu,
                    bias=b1sb[:, m:m + 1], scale=1.0,
                )

            h2t = h2pool.tile([P, K2, BLK], f32r, tag="h2t")
            for m in range(K2):
                pt = ps2.tile([P, BLK], f32, tag="ps2")
                for k in range(K1):
                    nc.tensor.matmul(
                        pt,
                        lhsT=w2sb[k][:, m * P:(m + 1) * P],
                        rhs=h1t[:, k, :],
                        start=(k == 0),
                        stop=(k == K1 - 1),
                    )
                # relu+bias on the vector engine to keep ScalarE headroom
                nc.vector.tensor_scalar(
                    out=h2t[:, m, :], in0=pt, scalar1=b2sb[:, m:m + 1],
                    scalar2=0.0, op0=Alu.add, op1=Alu.max,
                )

            # final layer: yT[1, BLK] = sum_k wf_k.T @ h2t_k  (fp32r, N=512)
            pty = psy.tile([1, BLK], f32, tag="psy")
            for k in range(K2):
                nc.tensor.matmul(
                    pty,
                    lhsT=wfsb[:, k:k + 1],
                    rhs=h2t[:, k, :],
                    start=(k == 0),
                    stop=(k == K2 - 1),
                )
            ytb = ytpool.tile([1, BLK], f32, tag="ytb")
            nc.scalar.activation(out=ytb, in_=pty, func=Act.Copy,
                                 bias=float(bf_val), scale=1.0)
            # scatter rows b*BLK..(b+1)*BLK into y2[p, c] with r = p*C + c
            if fast_scatter:
                nc.sync.dma_start(out=y2[b * PPB:(b + 1) * PPB, :],
                                  in_=ytb[0:1, :])
            else:
                nc.sync.dma_start(out=yt_dram[b * BLK:(b + 1) * BLK],
                                  in_=ytb[0:1, :])
            # every 4 blocks (engine partition offsets must be 32-aligned):
            # z0 = y*cinv - 1 and tau partial sums on the vector engine,
            # which is idle during the MLP — hides the projection preamble.
            if fast_scatter and (b + 1) * PPB % 32 == 0:
                pp = slice((b + 1) * PPB - 32, (b + 1) * PPB)
                nc.vector.tensor_tensor(out=z0[pp, :], in0=y2[pp, :],
                                        in1=ci_sb[pp, :], op=Alu.mult)
                nc.vector.tensor_scalar(out=z0[pp, :], in0=z0[pp, :],
                                        scalar1=-1.0, scalar2=None, op0=Alu.add)
                nc.vector.tensor_scalar(out=tmp[pp, :], in0=z0[pp, :],
                                        scalar1=EPS, scalar2=-EPS,
                                        op0=Alu.min, op1=Alu.max)
                if not no_pad:
                    nc.vector.tensor_tensor(out=tmp[pp, :], in0=tmp[pp, :],
                                            in1=a_sb[pp, :], op=Alu.mult)
                nc.vector.tensor_reduce(out=red[pp, :], in_=tmp[pp, :],
                                        axis=mybir.AxisListType.X, op=Alu.add)

        # ---- projection ----
        # z* = clip(z0 + lam*d, +-EPS) with scalar lam s.t.
        #   S(lam) := sum_r m_r clip(z0_r + lam*d_r)  ==  taun,
        #   taun = clip(S(0), +-DELTA*n)   (m absorbed in a2d when padded;
        #                                   m == 1 and plain sums when not)
        # 7-round 8-ary search over lam; state replicated across partitions,
        # each round = one DVE chain + one all-ones matmul (partition reduce).
        tmp3 = proj.tile([P, NCAND, C], f32, tag="tmp3")
        red7 = proj.tile([P, NCAND], f32, tag="red7")
        lamc = proj.tile([P, NCAND], f32, tag="lamc")
        predl = proj.tile([P, NCAND], f32, tag="predl")
        cnt = proj.tile([P, 1], f32, tag="cnt")
        taun = proj.tile([P, 1], f32, tag="taun")
        los = [proj.tile([P, 1], f32, tag=f"lo{i}", name=f"lo{i}")
               for i in range(2)]

        z0b = z0.rearrange("p (o c) -> p o c", o=1).to_broadcast([P, NCAND, C])
        d_b = d_sb.rearrange("p (o c) -> p o c", o=1).to_broadcast([P, NCAND, C])
        a_b = a_sb.rearrange("p (o c) -> p o c", o=1).to_broadcast([P, NCAND, C])

        if not fast_scatter:
            # general path: gather y back, then z0 / tau sums in one shot
            nc.sync.dma_start(out=y2, in_=yt_dram.rearrange("(p c) -> p c", p=P))
            nc.vector.tensor_tensor(out=z0, in0=y2, in1=ci_sb, op=Alu.mult)
            nc.vector.tensor_scalar(out=z0, in0=z0, scalar1=-1.0, scalar2=None,
                                    op0=Alu.add)
            nc.vector.tensor_scalar(out=tmp, in0=z0, scalar1=EPS, scalar2=-EPS,
                                    op0=Alu.min, op1=Alu.max)
            if not no_pad:
                nc.vector.tensor_tensor(out=tmp, in0=tmp, in1=a_sb, op=Alu.mult)
            nc.vector.tensor_reduce(out=red, in_=tmp,
                                    axis=mybir.AxisListType.X, op=Alu.add)
        # taun = clip(S(0), +-DELTA*n): partials in red were accumulated
        # per block during the MLP; finish with the partition reduce.
        t0ps = pst.tile([P, NCAND], f32, tag="hps", name="t0ps")
        nc.tensor.matmul(t0ps[:, 0:1], lhsT=onesmat, rhs=red, start=True,
                         stop=True)
        dn = DELTA * (float(nrows) if no_pad else 1.0)
        nc.vector.tensor_scalar(out=taun, in0=t0ps[:, 0:1], scalar1=dn,
                                scalar2=-dn, op0=Alu.min, op1=Alu.max)

        nc.vector.memset(los[0], -W0 / 2.0)
        lo = los[0]
        width = W0
        for r_i in range(N_ROUNDS):
            step = width / (NCAND + 1)
            # candidate lambdas: lo + j*step, j = 1..NCAND
            nc.vector.tensor_scalar(out=lamc, in0=iota, scalar1=step,
                                    scalar2=lo[:, 0:1], op0=Alu.mult,
                                    op1=Alu.add)
            lam_b = lamc.rearrange("p (o j) -> p j o", o=1).to_broadcast(
                [P, NCAND, C])
            nc.vector.tensor_tensor(out=tmp3, in0=d_b, in1=lam_b, op=Alu.mult)
            nc.vector.tensor_tensor(out=tmp3, in0=tmp3, in1=z0b, op=Alu.add)
            nc.vector.tensor_scalar(out=tmp3, in0=tmp3, scalar1=EPS,
                                    scalar2=-EPS, op0=Alu.min, op1=Alu.max)
            if not no_pad:
                nc.vector.tensor_tensor(out=tmp3, in0=tmp3, in1=a_b,
                                        op=Alu.mult)
            nc.vector.tensor_reduce(out=red7, in_=tmp3,
                                    axis=mybir.AxisListType.X, op=Alu.add)
            hps = pst.tile([P, NCAND], f32, tag="hps", name=f"hps{r_i}")
            nc.tensor.matmul(hps, lhsT=onesmat, rhs=red7, start=True,
                             stop=True)
            nc.vector.tensor_scalar(out=predl, in0=hps, scalar1=taun[:, 0:1],
                                    scalar2=None, op0=Alu.is_le)
            nc.vector.tensor_reduce(out=cnt, in_=predl,
                                    axis=mybir.AxisListType.X, op=Alu.add)
            lo_next = los[(r_i + 1) % 2]
            nc.vector.tensor_scalar(out=lo_next, in0=cnt, scalar1=step,
                                    scalar2=lo[:, 0:1], op0=Alu.mult,
                                    op1=Alu.add)
            lo = lo_next
            width = step  # interval shrinks to one candidate slot

        # final: lam = lo + width/2 ; out = (clip(z0 + lam*d) + 1) * c
        lamf = proj.tile([P, 1], f32, tag="lamf")
        nc.vector.tensor_scalar(out=lamf, in0=lo, scalar1=width / 2.0,
                                scalar2=None, op0=Alu.add)
        nc.vector.tensor_scalar(out=tmp, in0=d_sb, scalar1=lamf[:, 0:1],
                                scalar2=None, op0=Alu.mult)
        nc.vector.tensor_tensor(out=tmp, in0=tmp, in1=z0, op=Alu.add)
        nc.vector.tensor_scalar(out=tmp, in0=tmp, scalar1=EPS, scalar2=-EPS,
                                op0=Alu.min, op1=Alu.max)
        nc.vector.tensor_scalar(out=tmp, in0=tmp, scalar1=1.0, scalar2=None,
                                op0=Alu.add)
        nc.vector.tensor_tensor(out=tmp, in0=tmp, in1=c_sb, op=Alu.mult)
        nc.sync.dma_start(out=out2d, in_=tmp)

    nc.compile()
    return nc


def kernel(**inputs):
    global LAST_RESULT
    x = np.ascontiguousarray(np.asarray(inputs["x"], dtype=np.float32))
    W1 = np.ascontiguousarray(np.asarray(inputs["W1"], dtype=np.float32))
    b1 = np.ascontiguousarray(np.asarray(inputs["b1"], dtype=np.float32))
    W2 = np.ascontiguousarray(np.asarray(inputs["W2"], dtype=np.float32))
    b2 = np.ascontiguousarray(np.asarray(inputs["b2"], dtype=np.float32))
    Wf = np.ascontiguousarray(np.asarray(inputs["Wf"], dtype=np.float32))
    bf = float(np.asarray(inputs["bf"], dtype=np.float32).reshape(-1)[0])
    c = np.ascontiguousarray(np.asarray(inputs["constraint_constant"], dtype=np.float32))
    gm = np.asarray(inputs["group_mask"], dtype=np.float32)

    N, D = x.shape
    H1 = W1.shape[1]
    H2 = W2.shape[1]
    G = gm.shape[0]
    assert G == 8, "this kernel shards one quantile group per core"
    assert D % P == 0 and H1 % P == 0 and H2 % P == 0 and Wf.shape[1] == 1

    g = np.argmax(gm, axis=0)
    sizes = np.bincount(g, minlength=G)
    R = int(-(-sizes.max() // BLK) * BLK)   # per-core padded rows
    C = R // P
    uniform = bool((sizes == sizes[0]).all() and sizes[0] == R)

    order = np.argsort(g, kind="stable")
    bounds = np.zeros(G + 1, np.int64)
    np.cumsum(sizes, out=bounds[1:])

    def to2d(vec):
        # local row r -> (p = r // C, col = r % C)
        return np.ascontiguousarray(vec.reshape(P, C))

    in_maps = []
    rows_per_core = []
    for j in range(G):
        rows = order[bounds[j]:bounds[j + 1]]
        nrows = rows.shape[0]
        rows_per_core.append(rows)

        xtj = np.zeros((D, R), np.float32)
        xtj[:, :nrows] = x[rows].T   # row gather (contiguous) then T-assign

        cj = np.ones(R, np.float32)
        cj[:nrows] = c[rows]
        cij = 1.0 / cj
        dj = np.zeros(R, np.float32)
        dj[:nrows] = cij[:nrows] * cij[:nrows]
        aj = np.zeros(R, np.float32)
        aj[:nrows] = 1.0 / nrows

        in_maps.append(dict(
            xt=xtj, w1=W1, w2=W2, wf=Wf.reshape(-1), b1=b1, b2=b2,
            c2d=to2d(cj), ci2d=to2d(cij), d2d=to2d(dj), a2d=to2d(aj),
        ))

    nrows_build = R if uniform else -1   # -1 -> general padded path
    key = (D, H1, H2, R, nrows_build, float(bf))
    nc = _PROGRAM_CACHE.get(key)
    if nc is None:
        nc = _build_program(D, H1, H2, R, nrows_build, float(bf))
        _PROGRAM_CACHE[key] = nc

    from concourse.bass_utils import run_bass_kernel_spmd
    trace = bool(int(os.environ.get("KERNEL_PROFILE", "0")))
    res = run_bass_kernel_spmd(nc, in_maps, list(range(G)), trace=trace)
    LAST_RESULT = res

    out = np.empty((N, 1), np.float32)
    for j in range(G):
        y2d = res.results[j]["out2d"]          # [128, C], row r = p*C + col
        yvec = y2d.reshape(-1)
        out[rows_per_core[j], 0] = yvec[:rows_per_core[j].shape[0]]
    return out


# revision 19
# speedup vs baseline: 1.0339x; 1.0016x over previous
"""ConstrainedMLP Trainium2 kernel.

Strategy
--------
Rows are sharded BY GROUP (the quantile groups of `group_mask`): there are
exactly 8 groups and 8 cores, so each core owns one group and the
constrained-projection step becomes fully core-local (no collectives).

Per core:
  MLP   : h1 = relu(x@W1+b1), h2 = relu(h1@W2+b2), y_u = h2@Wf+bf
          computed in "transposed activation" layout (h1T/h2T = [feat, rows])
          with fp32r matmuls so every matmul has a 512-wide moving operand.
  Proj  : the reference's 300-iteration Dykstra projection converges to the
          exact QP solution  z* = clip(z0 + lam/w, +-EPS)  with a single
          scalar lam per group chosen so the group mean of z* hits
          clip(mean(clip(z0)), +-DELTA).  lam is found with a 7-round
          8-ary search (7 candidates per round, 3 bits/round); all search
          state is replicated across the 128 partitions so each round is a
          short DVE chain plus one all-ones matmul for the cross-partition
          reduction.  (Validated vs the 300-iter reference: ~2e-6.)

Self-contained: only numpy + the concourse/bass runtime installed in the
environment. No files are read.
"""

import os
import numpy as np

EPS = 0.15
DELTA = 0.05
W0 = 16.0        # initial lambda interval: [-8, 8] (lam* <= 2.7, 3x margin)
N_ROUNDS = 6     # 8-ary search rounds -> final width 16/8^6 ~ 6.1e-5
NCAND = 7        # candidates per round
P = 128          # SBUF partitions
BLK = 512        # row block (moving-operand width)

_PROGRAM_CACHE = {}
LAST_RESULT = None  # test harness introspection (exec_time etc.)


def _build_program(D, H1, H2, R, nrows, bf_val):
    import concourse.bass as bass
    import concourse.tile as tile
    from concourse import bacc, mybir
    from contextlib import ExitStack

    f32 = mybir.dt.float32
    f32r = mybir.dt.float32r
    Alu = mybir.AluOpType
    Act = mybir.ActivationFunctionType

    KD = D // P       # contraction chunks for layer 1
    K1 = H1 // P      # h1 feature chunks
    K2 = H2 // P      # h2 feature chunks
    NB = R // BLK     # row blocks
    C = R // P        # columns of the [128, C] projection layout
    fast_scatter = (BLK % C == 0)
    PPB = BLK // C if fast_scatter else 0  # partitions covered per row block
    no_pad = (nrows == R)

    nc = bacc.Bacc("TRN2", target_bir_lowering=False, debug=False, num_devices=8)

    xt = nc.dram_tensor("xt", [D, R], f32, kind="ExternalInput").ap()
    w1 = nc.dram_tensor("w1", [D, H1], f32, kind="ExternalInput").ap()
    w2 = nc.dram_tensor("w2", [H1, H2], f32, kind="ExternalInput").ap()
    wf = nc.dram_tensor("wf", [H2], f32, kind="ExternalInput").ap()
    b1 = nc.dram_tensor("b1", [H1], f32, kind="ExternalInput").ap()
    b2 = nc.dram_tensor("b2", [H2], f32, kind="ExternalInput").ap()
    c2d = nc.dram_tensor("c2d", [P, C], f32, kind="ExternalInput").ap()
    ci2d = nc.dram_tensor("ci2d", [P, C], f32, kind="ExternalInput").ap()
    d2d = nc.dram_tensor("d2d", [P, C], f32, kind="ExternalInput").ap()
    a2d = nc.dram_tensor("a2d", [P, C], f32, kind="ExternalInput").ap()
    out2d = nc.dram_tensor("out2d", [P, C], f32, kind="ExternalOutput").ap()

    with tile.TileContext(nc) as tc, ExitStack() as ctx:
        consts = ctx.enter_context(tc.tile_pool(name="consts", bufs=1))
        xpool = ctx.enter_context(tc.tile_pool(name="xp", bufs=6))
        h1pool = ctx.enter_context(tc.tile_pool(name="h1p", bufs=3))
        h2pool = ctx.enter_context(tc.tile_pool(name="h2p", bufs=3))
        ytpool = ctx.enter_context(tc.tile_pool(name="ytp", bufs=2))
        ps1 = ctx.enter_context(tc.tile_pool(name="ps1", bufs=3, space="PSUM"))
        ps2 = ctx.enter_context(tc.tile_pool(name="ps2", bufs=3, space="PSUM"))
        psy = ctx.enter_context(tc.tile_pool(name="psy", bufs=1, space="PSUM"))
        pst = ctx.enter_context(tc.tile_pool(name="pst", bufs=1, space="PSUM"))
        proj = ctx.enter_context(tc.tile_pool(name="proj", bufs=1))
        if not fast_scatter:
            dramp = ctx.enter_context(tc.tile_pool(name="dramp", bufs=1,
                                                   space="DRAM"))
            yt_dram = dramp.tile([R], f32, name="yt_dram")

        onesmat = consts.tile([P, P], f32, tag="onesmat")
        nc.vector.memset(onesmat, 1.0)
        iota = consts.tile([P, NCAND], f32, tag="iota")
        for j in range(NCAND):
            nc.vector.memset(iota[:, j:j + 1], float(j + 1))
        # PE warmup: dummy matmuls with no data deps run during the startup
        # DMA window so the HAM clock-gate is already at 2.4GHz when the
        # first real matmul issues (saves the ~3.4us cold-clock ramp).
        warm_junk = proj.tile([P, 1], f32, tag="warm_junk")
        for wi in range(10):
            wps = ps1.tile([P, BLK], f32, tag="ps1", name=f"warm{wi}")
            nc.tensor.matmul(wps[:, 0:P], lhsT=onesmat, rhs=onesmat,
                             start=True, stop=True)
            if wi == 9:
                nc.scalar.activation(out=warm_junk, in_=wps[:, 0:1],
                                     func=Act.Copy, bias=0.0, scale=1.0)

        # ---- resident weights / constants ----
        # Single ordered HWDGE stream arranged so the first L1 matmuls can
        # start as early as possible: w1[0], block-0 x tiles, w1[1:], b1,
        # then layer-2 / projection constants, then the b>=1 x stream.
        w1sb = [consts.tile([P, H1], f32r, tag=f"w1_{k}", name=f"w1sb{k}")
                for k in range(KD)]
        xts0 = [xpool.tile([P, BLK], f32r, tag=f"x{k}", name=f"x0_{k}")
                for k in range(KD)]
        b1sb = consts.tile([P, K1], f32, tag="b1")
        nc.sync.dma_start(out=b1sb, in_=b1.rearrange("(m p) -> p m", p=P))
        # first column-half of each w1 chunk + block-0 x first: the first
        # four m-groups can start ~3us earlier than waiting for full w1
        H1h = H1 // 2
        for k in range(KD):
            nc.sync.dma_start(out=w1sb[k][:, 0:H1h],
                              in_=w1[k * P:(k + 1) * P, 0:H1h].bitcast(f32r))
            nc.sync.dma_start(out=xts0[k], in_=xt[k * P:(k + 1) * P, 0:BLK].bitcast(f32r))
        for k in range(KD):
            nc.sync.dma_start(out=w1sb[k][:, H1h:H1],
                              in_=w1[k * P:(k + 1) * P, H1h:H1].bitcast(f32r))

        w2sb = []
        for k in range(K1):
            t = consts.tile([P, H2], f32r, tag=f"w2_{k}")
            nc.sync.dma_start(out=t, in_=w2[k * P:(k + 1) * P, :].bitcast(f32r))
            w2sb.append(t)
        wfsb = consts.tile([P, K2], f32r, tag="wf")
        nc.sync.dma_start(out=wfsb, in_=wf.rearrange("(k p) -> p k", p=P).bitcast(f32r))
        b2sb = consts.tile([P, K2], f32, tag="b2")
        nc.sync.dma_start(out=b2sb, in_=b2.rearrange("(m p) -> p m", p=P))

        ci_sb = consts.tile([P, C], f32, tag="ci_sb")
        nc.gpsimd.dma_start(out=ci_sb, in_=ci2d)
        c_sb = consts.tile([P, C], f32, tag="c_sb")
        nc.gpsimd.dma_start(out=c_sb, in_=c2d)
        d_sb = consts.tile([P, C], f32, tag="d_sb")
        nc.gpsimd.dma_start(out=d_sb, in_=d2d)
        a_sb = consts.tile([P, C], f32, tag="a_sb")
        nc.gpsimd.dma_start(out=a_sb, in_=a2d)

        y2 = consts.tile([P, C], f32, tag="y2")
        z0 = proj.tile([P, C], f32, tag="z0")
        tmp = proj.tile([P, C], f32, tag="tmp")
        red = proj.tile([P, 1], f32, tag="red")

        # ---- MLP over row blocks ----
        for b in range(NB):
            cols = slice(b * BLK, (b + 1) * BLK)
            if b == 0:
                xts = xts0
            else:
                xts = []
                for k in range(KD):
                    t = xpool.tile([P, BLK], f32r, tag=f"x{k}", name=f"x{b}_{k}")
                    nc.sync.dma_start(out=t, in_=xt[k * P:(k + 1) * P, cols].bitcast(f32r))
                    xts.append(t)

            h1t = h1pool.tile([P, K1, BLK], f32r, tag="h1t")
            for m in range(K1):
                pt = ps1.tile([P, BLK], f32, tag="ps1")
                for k in range(KD):
                    nc.tensor.matmul(
                        pt,
                        lhsT=w1sb[k][:, m * P:(m + 1) * P],
                        rhs=xts[k][:, :],
                        start=(k == 0),
                        stop=(k == KD - 1),
                    )
                nc.scalar.activation(
                    out=h1t[:, m, :], in_=pt, func=Act.Relu,
                    bias=b1sb[:, m:m + 1], scale=1.0,
                )

            h2t = h2pool.tile([P, K2, BLK], f32r, tag="h2t")
            for m in range(K2):
                pt = ps2.tile([P, BLK], f32, tag="ps2")
                for k in range(K1):
                    nc.tensor.matmul(
                        pt,
                        lhsT=w2sb[k][:, m * P:(m + 1) * P],
                        rhs=h1t[:, k, :],
                        start=(k == 0),
                        stop=(k == K1 - 1),
                    )
                # relu+bias on the vector engine to keep ScalarE headroom
                nc.vector.tensor_scalar(
                    out=h2t[:, m, :], in0=pt, scalar1=b2sb[:, m:m + 1],
                    scalar2=0.0, op0=Alu.add, op1=Alu.max,
                )

            # final layer: yT[1, BLK] = sum_k wf_k.T @ h2t_k  (fp32r, N=512)
            pty = psy.tile([1, BLK], f32, tag="psy")
            for k in range(K2):
                nc.tensor.matmul(
                    pty,
                    lhsT=wfsb[:, k:k + 1],
                    rhs=h2t[:, k, :],
                    start=(k == 0),
                    stop=(k == K2 - 1),
                )
            ytb = ytpool.tile([1, BLK], f32, tag="ytb")
            nc.scalar.activation(out=ytb, in_=pty, func=Act.Copy,
                                 bias=float(bf_val), scale=1.0)
            # scatter rows b*BLK..(b+1)*BLK into y2[p, c] with r = p*C + c
            if fast_scatter:
                nc.gpsimd.dma_start(out=y2[b * PPB:(b + 1) * PPB, :],
                                    in_=ytb[0:1, :])
            else:
                nc.gpsimd.dma_start(out=yt_dram[b * BLK:(b + 1) * BLK],
                                    in_=ytb[0:1, :])
            # every 4 blocks (engine partition offsets must be 32-aligned):
            # z0 = y*cinv - 1 and tau partial sums on the vector engine,
            # which is idle during the MLP — hides the projection preamble.
            if fast_scatter and (b + 1) * PPB % 32 == 0:
                pp = slice((b + 1) * PPB - 32, (b + 1) * PPB)
                nc.vector.tensor_tensor(out=z0[pp, :], in0=y2[pp, :],
                                        in1=ci_sb[pp, :], op=Alu.mult)
                nc.vector.tensor_scalar(out=z0[pp, :], in0=z0[pp, :],
                                        scalar1=-1.0, scalar2=None, op0=Alu.add)
                nc.vector.tensor_scalar(out=tmp[pp, :], in0=z0[pp, :],
                                        scalar1=EPS, scalar2=-EPS,
                                        op0=Alu.min, op1=Alu.max)
                if not no_pad:
                    nc.vector.tensor_tensor(out=tmp[pp, :], in0=tmp[pp, :],
                                            in1=a_sb[pp, :], op=Alu.mult)
                nc.vector.tensor_reduce(out=red[pp, :], in_=tmp[pp, :],
                                        axis=mybir.AxisListType.X, op=Alu.add)

        # ---- projection ----
        # z* = clip(z0 + lam*d, +-EPS) with scalar lam s.t.
        #   S(lam) := sum_r m_r clip(z0_r + lam*d_r)  ==  taun,
        #   taun = clip(S(0), +-DELTA*n)   (m absorbed in a2d when padded;
        #                                   m == 1 and plain sums when not)
        # 7-round 8-ary search over lam; state replicated across partitions,
        # each round = one DVE chain + one all-ones matmul (partition reduce).
        tmp3 = proj.tile([P, NCAND, C], f32, tag="tmp3")
        red7 = proj.tile([P, NCAND], f32, tag="red7")
        lamc = proj.tile([P, NCAND], f32, tag="lamc")
        predl = proj.tile([P, NCAND], f32, tag="predl")
        cnt = proj.tile([P, 1], f32, tag="cnt")
        taun = proj.tile([P, 1], f32, tag="taun")
        los = [proj.tile([P, 1], f32, tag=f"lo{i}", name=f"lo{i}")
               for i in range(2)]

        z0b = z0.rearrange("p (o c) -> p o c", o=1).to_broadcast([P, NCAND, C])
        d_b = d_sb.rearrange("p (o c) -> p o c", o=1).to_broadcast([P, NCAND, C])
        a_b = a_sb.rearrange("p (o c) -> p o c", o=1).to_broadcast([P, NCAND, C])

        if not fast_scatter:
            # general path: gather y back, then z0 / tau sums in one shot
            nc.sync.dma_start(out=y2, in_=yt_dram.rearrange("(p c) -> p c", p=P))
            nc.vector.tensor_tensor(out=z0, in0=y2, in1=ci_sb, op=Alu.mult)
            nc.vector.tensor_scalar(out=z0, in0=z0, scalar1=-1.0, scalar2=None,
                                    op0=Alu.add)
            nc.vector.tensor_scalar(out=tmp, in0=z0, scalar1=EPS, scalar2=-EPS,
                                    op0=Alu.min, op1=Alu.max)
            if not no_pad:
                nc.vector.tensor_tensor(out=tmp, in0=tmp, in1=a_sb, op=Alu.mult)
            nc.vector.tensor_reduce(out=red, in_=tmp,
                                    axis=mybir.AxisListType.X, op=Alu.add)
        # taun = clip(S(0), +-DELTA*n): partials in red were accumulated
        # per block during the MLP; finish with the partition reduce.
        t0ps = pst.tile([P, NCAND], f32, tag="hps", name="t0ps")
        nc.tensor.matmul(t0ps[:, 0:1], lhsT=onesmat, rhs=red, start=True,
                         stop=True)
        dn = DELTA * (float(nrows) if no_pad else 1.0)
        nc.vector.tensor_scalar(out=taun, in0=t0ps[:, 0:1], scalar1=dn,
                                scalar2=-dn, op0=Alu.min, op1=Alu.max)

        nc.vector.memset(los[0], -W0 / 2.0)
        lo = los[0]
        width = W0
        for r_i in range(N_ROUNDS):
            step = width / (NCAND + 1)
            # candidate lambdas: lo + j*step, j = 1..NCAND
            nc.vector.tensor_scalar(out=lamc, in0=iota, scalar1=step,
                                    scalar2=lo[:, 0:1], op0=Alu.mult,
                                    op1=Alu.add)
            lam_b = lamc.rearrange("p (o j) -> p j o", o=1).to_broadcast(
                [P, NCAND, C])
            nc.vector.tensor_tensor(out=tmp3, in0=d_b, in1=lam_b, op=Alu.mult)
            nc.vector.tensor_tensor(out=tmp3, in0=tmp3, in1=z0b, op=Alu.add)
            nc.vector.tensor_scalar(out=tmp3, in0=tmp3, scalar1=EPS,
                                    scalar2=-EPS, op0=Alu.min, op1=Alu.max)
            if not no_pad:
                nc.vector.tensor_tensor(out=tmp3, in0=tmp3, in1=a_b,
                                        op=Alu.mult)
            nc.vector.tensor_reduce(out=red7, in_=tmp3,
                                    axis=mybir.AxisListType.X, op=Alu.add)
            hps = pst.tile([P, NCAND], f32, tag="hps", name=f"hps{r_i}")
            nc.tensor.matmul(hps, lhsT=onesmat, rhs=red7, start=True,
                             stop=True)
            nc.vector.tensor_scalar(out=predl, in0=hps, scalar1=taun[:, 0:1],
                                    scalar2=None, op0=Alu.is_le)
            nc.vector.tensor_reduce(out=cnt, in_=predl,
                                    axis=mybir.AxisListType.X, op=Alu.add)
            lo_next = los[(r_i + 1) % 2]
            nc.vector.tensor_scalar(out=lo_next, in0=cnt, scalar1=step,
                                    scalar2=lo[:, 0:1], op0=Alu.mult,
                                    op1=Alu.add)
            lo = lo_next
            width = step  # interval shrinks to one candidate slot

        # final: lam = lo + width/2 ; out = (clip(z0 + lam*d) + 1) * c
        lamf = proj.tile([P, 1], f32, tag="lamf")
        nc.vector.tensor_scalar(out=lamf, in0=lo, scalar1=width / 2.0,
                                scalar2=None, op0=Alu.add)
        nc.vector.tensor_scalar(out=tmp, in0=d_sb, scalar1=lamf[:, 0:1],
                                scalar2=None, op0=Alu.mult)
        nc.vector.tensor_tensor(out=tmp, in0=tmp, in1=z0, op=Alu.add)
        nc.vector.tensor_scalar(out=tmp, in0=tmp, scalar1=EPS, scalar2=-EPS,
                                op0=Alu.min, op1=Alu.max)
        nc.vector.tensor_scalar(out=tmp, in0=tmp, scalar1=1.0, scalar2=None,
                                op0=Alu.add)
        nc.vector.tensor_tensor(out=tmp, in0=tmp, in1=c_sb, op=Alu.mult)
        nc.sync.dma_start(out=out2d, in_=tmp)

    nc.compile()
    return nc


def kernel(**inputs):
    global LAST_RESULT
    x = np.ascontiguousarray(np.asarray(inputs["x"], dtype=np.float32))
    W1 = np.ascontiguousarray(np.asarray(inputs["W1"], dtype=np.float32))
    b1 = np.ascontiguousarray(np.asarray(inputs["b1"], dtype=np.float32))
    W2 = np.ascontiguousarray(np.asarray(inputs["W2"], dtype=np.float32))
    b2 = np.ascontiguousarray(np.asarray(inputs["b2"], dtype=np.float32))
    Wf = np.ascontiguousarray(np.asarray(inputs["Wf"], dtype=np.float32))
    bf = float(np.asarray(inputs["bf"], dtype=np.float32).reshape(-1)[0])
    c = np.ascontiguousarray(np.asarray(inputs["constraint_constant"], dtype=np.float32))
    gm = np.asarray(inputs["group_mask"], dtype=np.float32)

    N, D = x.shape
    H1 = W1.shape[1]
    H2 = W2.shape[1]
    G = gm.shape[0]
    assert G == 8, "this kernel shards one quantile group per core"
    assert D % P == 0 and H1 % P == 0 and H2 % P == 0 and Wf.shape[1] == 1

    g = np.argmax(gm, axis=0)
    sizes = np.bincount(g, minlength=G)
    R = int(-(-sizes.max() // BLK) * BLK)   # per-core padded rows
    C = R // P
    uniform = bool((sizes == sizes[0]).all() and sizes[0] == R)

    order = np.argsort(g, kind="stable")
    bounds = np.zeros(G + 1, np.int64)
    np.cumsum(sizes, out=bounds[1:])

    def to2d(vec):
        # local row r -> (p = r // C, col = r % C)
        return np.ascontiguousarray(vec.reshape(P, C))

    in_maps = []
    rows_per_core = []
    for j in range(G):
        rows = order[bounds[j]:bounds[j + 1]]
        nrows = rows.shape[0]
        rows_per_core.append(rows)

        xtj = np.zeros((D, R), np.float32)
        xtj[:, :nrows] = x[rows].T   # row gather (contiguous) then T-assign

        cj = np.ones(R, np.float32)
        cj[:nrows] = c[rows]
        cij = 1.0 / cj
        dj = np.zeros(R, np.float32)
        dj[:nrows] = cij[:nrows] * cij[:nrows]
        aj = np.zeros(R, np.float32)
        aj[:nrows] = 1.0 / nrows

        in_maps.append(dict(
            xt=xtj, w1=W1, w2=W2, wf=Wf.reshape(-1), b1=b1, b2=b2,
            c2d=to2d(cj), ci2d=to2d(cij), d2d=to2d(dj), a2d=to2d(aj),
        ))

    nrows_build = R if uniform else -1   # -1 -> general padded path
    key = (D, H1, H2, R, nrows_build, float(bf))
    nc = _PROGRAM_CACHE.get(key)
    if nc is None:
        nc = _build_program(D, H1, H2, R, nrows_build, float(bf))
        _PROGRAM_CACHE[key] = nc

    from concourse.bass_utils import run_bass_kernel_spmd
    trace = bool(int(os.environ.get("KERNEL_PROFILE", "0")))
    res = run_bass_kernel_spmd(nc, in_maps, list(range(G)), trace=trace)
    LAST_RESULT = res

    out = np.empty((N, 1), np.float32)
    for j in range(G):
        y2d = res.results[j]["out2d"]          # [128, C], row r = p*C + col
        yvec = y2d.reshape(-1)
        out[rows_per_core[j], 0] = yvec[:rows_per_core[j].shape[0]]
    return out
